# revision 32
# baseline (speedup 1.0000x reference)
"""Distributed multi-head attention kernel for 8 TRN2 NeuronCores.

Sharding: tensor-parallel over heads (2 heads/core). Per core: qkv projection
for its 128 features, attention for its 2 heads, AllToAll exchange, then
row-parallel output projection (each core produces a transposed 512-row slice
of the final output); host reassembles.

Structure:
- V is projected directly in [rows, feat] layout (stationary = x k-tile,
  M = 128 rows): no PE transposes anywhere.
- PV uses column-split tile_position packing: both heads' PV matmuls run
  concurrently in column halves of the PE array (one 512-cycle pass per key
  tile instead of two M=65 passes).
- Softmax denominators come from a bf16 pair/quad/hex DVE reduction tree over
  the exp tiles plus 4 all-ones matmuls per block; reciprocals via the DVE
  reciprocal_approx_fast custom op (the scalar engine runs nothing but the
  128 exp tiles); normalization is two plain DVE multiplies whose recip
  operand layout matches po's partition split.
- The attention phase is ACT(exp)-bound, so everything else is threaded
  through its PE slack: the emission is software-pipelined (QK/exp of tile
  kt runs SHIFT tiles ahead of PV), the previous block's denominator/
  normalize work is spread over the next block's first tiles, and the whole
  remaining projection work (q for later blocks, all of batch 1) is emitted
  as fine-grained units popped between key tiles with deadline ordering.
- The exchange is split into two half-query AllToAlls so the first half's
  output projection overlaps the second collective (which pays no extra
  cross-core skew: the first collective already synced the cores); the
  output projection is k-outer so it starts as soon as the first received
  k-tile lands, with bias+store pipelined into the last k-slice.

Compute in bf16 on the PE array (f32 PSUM accumulation, f32 softmax
denominators/normalization). The host pre-transposes x to [dim, b*s] and
pre-casts x/wqkv/wo to bf16 as part of sharding/layout prep.
"""

import sys

sys.path.insert(0, "/opt/trn_rl_repo")

import ml_dtypes
import numpy as np

# Problem constants (hardcoded per harness contract)
B = 2
S = 2048
DIM = 1024
N_HEAD = 16
HD = 64  # head dim
SCALE = HD ** (-0.5)
R = B * S  # 4096 flattened rows
NCORES = 8
HPC = N_HEAD // NCORES  # 2 heads per core
FPC = HPC * HD  # 128 features per core
RPC = R // NCORES  # 512 rows per core (output row slice)

KT = DIM // 128  # 8 k-tiles over the model dim
NKT = S // 128  # 16 key tiles per sequence
NQB = S // 512  # 4 query blocks per sequence
SHIFT = 3  # PV pipeline lag behind QK/exp

_CACHED = {}


def _build_graph():
    import concourse.mybir as mybir
    import concourse.tile as tile
    from concourse import bacc

    nc = bacc.Bacc(
        "TRN2",
        target_bir_lowering=False,
        debug=False,
        num_devices=NCORES,
    )
    return _build_body(nc, mybir, tile)


def _build_body(nc, mybir, tile):
    f32 = mybir.dt.float32
    bf16 = mybir.dt.bfloat16
    EXP = mybir.ActivationFunctionType.Exp

    xt = nc.dram_tensor("xt", [DIM, R], bf16, kind="ExternalInput").ap()
    wqkv = nc.dram_tensor("wqkv", [DIM, 3 * FPC], bf16, kind="ExternalInput").ap()
    bqkv = nc.dram_tensor("bqkv", [3, FPC], f32, kind="ExternalInput").ap()
    wo = nc.dram_tensor("wo", [DIM, DIM], bf16, kind="ExternalInput").ap()
    bo = nc.dram_tensor("bo", [8, 128], f32, kind="ExternalInput").ap()
    out = nc.dram_tensor("out", [DIM, RPC], bf16, kind="ExternalOutput").ap()

    with tile.TileContext(nc) as tc:
        with (
            tc.tile_pool(name="glob", bufs=1) as glob,
            tc.tile_pool(name="dram", bufs=1, space="DRAM") as dram_pool,
        ):
            # ---------------- persistent tiles -------------------------
            ones128 = glob.tile([128, 128], bf16)
            nc.vector.memset(ones128[:], 1.0)
            bias_qkv = glob.tile([128, 2], f32)  # q, k per-partition biases
            vbias = glob.tile([128, 128], f32)  # v bias along free dim
            bias_o = glob.tile([128, 8], f32)
            qT = glob.tile([128, R], bf16)
            kT = glob.tile([128, R], bf16)
            v_nat = glob.tile([128, R], bf16)  # [keys, 2h*64d] per 128-chunk

            warm_in = dram_pool.tile([NCORES, 16], bf16, name="warm_in")
            warm_out = dram_pool.tile([NCORES, 16], bf16, name="warm_out")
            a2a_inA = dram_pool.tile([DIM, RPC // 2], bf16, name="a2a_inA")
            a2a_outA = dram_pool.tile([DIM, RPC // 2], bf16, name="a2a_outA")
            a2a_inB = dram_pool.tile([DIM, RPC // 2], bf16, name="a2a_inB")
            a2a_outB = dram_pool.tile([DIM, RPC // 2], bf16, name="a2a_outB")

            # ---------------- phase 0: weight/bias DMAs, warm a2a ------
            wqkv_sb = []
            for k in range(KT):
                w_t = glob.tile([128, 3 * FPC], bf16, name=f"w_{k}")
                nc.gpsimd.dma_start(out=w_t[:], in_=wqkv[k * 128 : (k + 1) * 128, :])
                wqkv_sb.append(w_t)
            for m in range(2):
                nc.gpsimd.dma_start(
                    out=bias_qkv[:, m : m + 1], in_=bqkv[m : m + 1, :]
                )
            nc.gpsimd.dma_start(
                out=vbias[:], in_=bqkv[2:3, :].to_broadcast((128, 128))
            )
            for m in range(8):
                nc.gpsimd.dma_start(out=bias_o[:, m : m + 1], in_=bo[m : m + 1, :])

            warm_sb = glob.tile([1, 16], bf16)
            nc.vector.memset(warm_sb[:], 1.0)
            nc.gpsimd.dma_start(out=warm_in[0:1, :], in_=warm_sb[0:1, :])
            nc.gpsimd.dma_start(
                out=warm_in[1:NCORES, :],
                in_=warm_in[0:1, :].to_broadcast((NCORES - 1, 16)),
            )
            nc.gpsimd.collective_compute(
                "AllToAll",
                mybir.AluOpType.bypass,
                replica_groups=[list(range(NCORES))],
                ins=[warm_in[:].opt()],
                outs=[warm_out[:].opt()],
            )
            wo_sb = []
            for k in range(KT):
                w_t = glob.tile([128, DIM], bf16, name=f"wo_{k}")
                nc.gpsimd.dma_start(out=w_t[:], in_=wo[k * 128 : (k + 1) * 128, :])
                wo_sb.append(w_t)

            with tc.tile_pool(name="xTp", bufs=2) as xT_pool:

                def dma_group(g, eng=None, split=False):
                    """DMA one 1024-row group of xt; returns the 8 k-tiles."""
                    eng = eng or nc.sync
                    xg = []
                    for k in range(KT):
                        t = xT_pool.tile(
                            [128, 1024], bf16, name=f"xT_{k}", tag=f"xT{k}"
                        )
                        e = nc.gpsimd if (split and k % 2) else eng
                        e.dma_start(
                            out=t[:],
                            in_=xt[
                                k * 128 : (k + 1) * 128, g * 1024 : (g + 1) * 1024
                            ],
                        )
                        xg.append(t)
                    return xg

                def qk_mms(pp, xg, m, h, ks):
                    for k in ks:
                        nc.tensor.matmul(
                            pp[:],
                            lhsT=wqkv_sb[k][:, m * 128 : (m + 1) * 128],
                            rhs=xg[k][:, h * 512 : (h + 1) * 512],
                            start=(k == 0),
                            stop=(k == KT - 1),
                        )

                def qk_bias(pp, g, m, h):
                    col0 = g * 1024 + h * 512
                    dst = qT if m == 0 else kT
                    nc.vector.tensor_scalar_add(
                        out=dst[:, col0 : col0 + 512],
                        in0=pp[:],
                        scalar1=bias_qkv[:, m : m + 1],
                    )

                def v_mms(vd, xg, c, ks):
                    for k in ks:
                        nc.tensor.matmul(
                            vd[:],
                            lhsT=xg[k][:, c * 128 : (c + 1) * 128],
                            rhs=wqkv_sb[k][:, 256:384],
                            start=(k == 0),
                            stop=(k == KT - 1),
                        )

                def v_bias(vd, g, c):
                    chunk = g * 8 + c
                    nc.vector.tensor_add(
                        out=v_nat[:, chunk * 128 : (chunk + 1) * 128],
                        in0=vd[:],
                        in1=vbias[:],
                    )

                # -------- phase 1 prefix: just enough to start block 0 --
                xgs = {}
                with (
                    tc.tile_pool(name="pp1", bufs=2, space="PSUM") as pp1_pool,
                    tc.tile_pool(name="vd1", bufs=1, space="PSUM") as vd1_pool,
                ):
                    xgs[0] = dma_group(0)
                    xgs[1] = dma_group(1)
                    # mini k-round: key tile 0 only, so the first QK/exp can
                    # issue several microseconds before the full rounds land
                    ppm = pp1_pool.tile([128, 128], f32, name="ppm", tag="pp")
                    for k in range(KT):
                        nc.tensor.matmul(
                            ppm[:],
                            lhsT=wqkv_sb[k][:, 128:256],
                            rhs=xgs[0][k][:, 0:128],
                            start=(k == 0),
                            stop=(k == KT - 1),
                        )
                    nc.vector.tensor_scalar_add(
                        out=kT[:, 0:128], in0=ppm[:], scalar1=bias_qkv[:, 1:2]
                    )
                    pp = pp1_pool.tile([128, 512], f32, name="pp", tag="pp")
                    qk_mms(pp, xgs[0], 0, 0, range(KT))  # q, rows 0-511
                    qk_bias(pp, 0, 0, 0)
                    pp = pp1_pool.tile([128, 512], f32, name="pp", tag="pp")
                    qk_mms(pp, xgs[0], 1, 0, range(KT))  # k, rows 0-511
                    qk_bias(pp, 0, 1, 0)
                    vd = vd1_pool.tile([128, 128], f32, name="vd", tag="vd")
                    v_mms(vd, xgs[0], 0, range(KT))  # v chunk 0
                    v_bias(vd, 0, 0)

                # -------- phase 2: attention + interleaved projection --
                with (
                    tc.tile_pool(name="pstp", bufs=2, space="PSUM") as pst_pool,
                    tc.tile_pool(name="pop", bufs=1, space="PSUM") as po_pool,
                    tc.tile_pool(name="denp", bufs=1, space="PSUM") as den_pool,
                    tc.tile_pool(name="pp2", bufs=1, space="PSUM") as pp2_pool,
                    tc.tile_pool(name="ptp", bufs=12) as pt_pool,
                    tc.tile_pool(name="pairp", bufs=2) as pair_pool,
                    tc.tile_pool(name="quadp", bufs=2) as quad_pool,
                    tc.tile_pool(name="hexp", bufs=2) as hex_pool,
                    tc.tile_pool(name="recipp", bufs=2) as recip_pool,
                    tc.tile_pool(name="oTsp", bufs=2) as oTs_pool,
                ):
                    st = {"pp": None, "vd": None, "pending": None}

                    # ---- deferred projection units (deadline-ordered) --
                    def u_dma(g):
                        return lambda: xgs.__setitem__(g, dma_group(g, nc.gpsimd))

                    def u_round_start(g, m, h, pool):
                        def f():
                            st["pp"] = pool.tile(
                                [128, 512], f32, name="pp", tag="pp"
                            )
                            qk_mms(st["pp"], xgs[g], m, h, range(2))

                        return f

                    def u_round_mid(g, m, h, ks):
                        return lambda: qk_mms(st["pp"], xgs[g], m, h, ks)

                    def u_round_end(g, m, h):
                        def f():
                            qk_mms(st["pp"], xgs[g], m, h, range(6, 8))
                            qk_bias(st["pp"], g, m, h)

                        return f

                    def round_units(g, m, h, pool):
                        return [
                            u_round_start(g, m, h, pool),
                            u_round_mid(g, m, h, range(2, 4)),
                            u_round_mid(g, m, h, range(4, 6)),
                            u_round_end(g, m, h),
                        ]

                    def u_v_a(g, c, pool):
                        def f():
                            st["vd"] = pool.tile(
                                [128, 128], f32, name="vd", tag="pp"
                            )
                            v_mms(st["vd"], xgs[g], c, range(4))

                        return f

                    def u_v_b(g, c):
                        def f():
                            v_mms(st["vd"], xgs[g], c, range(4, 8))
                            v_bias(st["vd"], g, c)

                        return f

                    def v_units(g, c, pool):
                        return [u_v_a(g, c, pool), u_v_b(g, c)]

                    p2 = pp2_pool
                    units = []
                    # batch-0 remainder, deadline-interleaved for block 0
                    # (3 pops/kt): v chunk c is needed by PV at kt c+SHIFT;
                    # k-round (g,h) covers key tiles g*8+h*4 .. +3.
                    units += v_units(0, 1, p2) + v_units(0, 2, p2)
                    units += round_units(0, 1, 1, p2)  # k rows 512-1023
                    units += v_units(0, 3, p2) + v_units(0, 4, p2)
                    units += round_units(1, 1, 0, p2)  # k rows 1024-1535
                    units += v_units(0, 5, p2) + v_units(0, 6, p2)
                    units += round_units(1, 1, 1, p2)  # k rows 1536-2047
                    units += v_units(0, 7, p2)
                    for c in range(8):
                        units += v_units(1, c, p2)  # v rows 1024-2047
                    units += round_units(0, 0, 1, p2)  # q for block 1
                    units += round_units(1, 0, 0, p2)  # q for block 2
                    units += round_units(1, 0, 1, p2)  # q for block 3
                    # batch 1: k and v (needed by block 4), q(g2,h0) too
                    units += [u_dma(2)]
                    units += round_units(2, 1, 0, p2) + round_units(2, 1, 1, p2)
                    for c in range(4):
                        units += v_units(2, c, p2)
                    units += [u_dma(3)]
                    for c in range(4, 8):
                        units += v_units(2, c, p2)
                    units += round_units(3, 1, 0, p2) + round_units(3, 1, 1, p2)
                    for c in range(8):
                        units += v_units(3, c, p2)
                    units += round_units(2, 0, 0, p2)  # q for block 4
                    # popped during blocks 4-6:
                    late_units = (
                        round_units(2, 0, 1, p2)  # q block 5
                        + round_units(3, 0, 0, p2)  # q block 6
                        + round_units(3, 0, 1, p2)  # q block 7
                    )
                    units.reverse()
                    late_units.reverse()

                    def emit_pv(blk, kt, pts, po, tree):
                        b = blk // NQB
                        off = (b * NKT + kt) * 128
                        pt = pts[kt]
                        nc.tensor.matmul(
                            po[0:64, :],
                            lhsT=v_nat[:, off : off + 64],
                            rhs=pt[:, 0:512],
                            start=(kt == 0),
                            stop=(kt == NKT - 1),
                            tile_position=(0, 0),
                        )
                        nc.tensor.matmul(
                            po[64:128, :],
                            lhsT=v_nat[:, off + 64 : off + 128],
                            rhs=pt[:, 512:1024],
                            start=(kt == 0),
                            stop=(kt == NKT - 1),
                            tile_position=(0, 64),
                        )
                        # bf16 reduction tree toward the denominators
                        if kt % 2 == 1:
                            pr = pair_pool.tile(
                                [128, 1024], bf16, name="pair", tag="pair"
                            )
                            nc.vector.tensor_add(
                                out=pr[:], in0=pts[kt - 1][:], in1=pt[:]
                            )
                            tree["pair"].append(pr)
                        if kt % 4 == 3:
                            qd = quad_pool.tile(
                                [128, 1024], bf16, name="quad", tag="quad"
                            )
                            nc.vector.tensor_add(
                                out=qd[:],
                                in0=tree["pair"][-2][:],
                                in1=tree["pair"][-1][:],
                            )
                            tree["quad"].append(qd)
                        if kt % 8 == 7:
                            hx = hex_pool.tile(
                                [128, 1024], bf16, name="hex", tag="hex"
                            )
                            nc.vector.tensor_add(
                                out=hx[:],
                                in0=tree["quad"][-2][:],
                                in1=tree["quad"][-1][:],
                            )
                            tree["hex"].append(hx)

                    def tail_a1(blk, pts, po, tree):
                        emit_pv(blk, NKT - 3, pts, po, tree)

                    def tail_a2(blk, pts, po, tree):
                        emit_pv(blk, NKT - 2, pts, po, tree)
                        emit_pv(blk, NKT - 1, pts, po, tree)

                    def tail_b(blk, pts, po, tree):
                        # denominator part 1: hex0 into both halves (start)
                        dn = den_pool.tile([128, 1024], f32, name="den", tag="den")
                        st["den"] = dn
                        hx0 = tree["hex"][0]
                        for half in range(2):
                            c0 = half * 512
                            nc.tensor.matmul(
                                dn[:, c0 : c0 + 512],
                                lhsT=ones128[:],
                                rhs=hx0[:, c0 : c0 + 512],
                                start=True,
                                stop=False,
                            )

                    def tail_c(blk, pts, po, tree):
                        dn = st["den"]
                        hx1 = tree["hex"][1]
                        for half in range(2):
                            c0 = half * 512
                            nc.tensor.matmul(
                                dn[:, c0 : c0 + 512],
                                lhsT=ones128[:],
                                rhs=hx1[:, c0 : c0 + 512],
                                start=False,
                                stop=True,
                            )
                        recip = recip_pool.tile(
                            [128, 1024], f32, name="recip", tag="rc"
                        )
                        nc.vector.reciprocal_approx_fast(out=recip[:], in_=dn[:])
                        oTs = oTs_pool.tile([128, 512], bf16, name="oTs", tag="oTs")
                        nc.vector.tensor_mul(
                            out=oTs[0:64, :],
                            in0=po[0:64, :],
                            in1=recip[0:64, 0:512],
                        )
                        nc.vector.tensor_mul(
                            out=oTs[64:128, :],
                            in0=po[64:128, :],
                            in1=recip[64:128, 512:1024],
                        )
                        nc.sync.dma_start(
                            out=a2a_inA[blk * 128 : (blk + 1) * 128, :],
                            in_=oTs[:, 0 : RPC // 2],
                        )
                        nc.sync.dma_start(
                            out=a2a_inB[blk * 128 : (blk + 1) * 128, :],
                            in_=oTs[:, RPC // 2 : RPC],
                        )

                    for b in range(B):
                        for qb in range(NQB):
                            blk = b * NQB + qb
                            q0 = b * S + qb * 512
                            pts = []
                            tree = {"pair": [], "quad": [], "hex": []}
                            po = None
                            for kt in range(NKT):
                                k0 = b * S + kt * 128
                                pst = pst_pool.tile(
                                    [128, 1024], f32, name="pst", tag="st"
                                )
                                for hh in range(HPC):
                                    nc.tensor.matmul(
                                        pst[:, hh * 512 : (hh + 1) * 512],
                                        lhsT=kT[
                                            hh * 64 : (hh + 1) * 64, k0 : k0 + 128
                                        ],
                                        rhs=qT[
                                            hh * 64 : (hh + 1) * 64, q0 : q0 + 512
                                        ],
                                        start=True,
                                        stop=True,
                                        tile_position=(hh * 64, 0),
                                    )
                                pt = pt_pool.tile(
                                    [128, 1024], bf16, name="ptile", tag="pt"
                                )
                                nc.scalar.activation(
                                    pt[:], pst[:], EXP, scale=SCALE
                                )
                                pts.append(pt)
                                pend = st["pending"]
                                if kt == 0 and pend:
                                    tail_a1(*pend)
                                elif kt == 1 and pend:
                                    tail_a2(*pend)
                                elif kt == 2 and pend:
                                    tail_b(*pend)
                                elif kt == 3 and pend:
                                    tail_c(*pend)
                                    st["pending"] = None
                                if kt == SHIFT:
                                    po = po_pool.tile(
                                        [128, 512], f32, name="po", tag="po"
                                    )
                                if kt >= SHIFT:
                                    emit_pv(blk, kt - SHIFT, pts, po, tree)
                                # deadline-paced unit pops, kept away from the
                                # block-boundary key-tiles that carry the
                                # previous block's denominator/normalize work
                                npop = 0
                                if blk == 0:
                                    npop = 3 if kt < 14 else 2
                                elif blk < 4:
                                    npop = 0 if kt < 4 else (2 if kt < 14 else 1)
                                elif blk < 7:
                                    npop = 1 if kt in (5, 7, 9, 11) else 0
                                for _ in range(npop):
                                    if blk < 4 and units:
                                        units.pop()()
                                    elif late_units:
                                        late_units.pop()()
                            st["pending"] = (blk, pts, po, tree)
                    # flush the last block, then keep-warm + exchange
                    tail_a1(*st["pending"])
                    tail_a2(*st["pending"])
                    tail_b(*st["pending"])
                    tail_c(*st["pending"])
                    st["pending"] = None
                    while units:
                        units.pop()()
                    while late_units:
                        late_units.pop()()
                    for buf_in, buf_out in ((a2a_inA, a2a_outA), (a2a_inB, a2a_outB)):
                        nc.gpsimd.collective_compute(
                            "AllToAll",
                            mybir.AluOpType.bypass,
                            replica_groups=[list(range(NCORES))],
                            ins=[buf_in[:].opt()],
                            outs=[buf_out[:].opt()],
                        )

            # ---------------- phase 3: output projection ---------------
            with (
                tc.tile_pool(name="ots", bufs=1) as ots_pool,
                tc.tile_pool(name="psout", bufs=1, space="PSUM") as ps_out,
                tc.tile_pool(name="outt", bufs=2) as out_pool,
            ):
                pouts = [
                    ps_out.tile([128, 256], f32, name=f"pout{m}", tag=f"po{m}")
                    for m in range(8)
                ]
                for half, buf_out in ((0, a2a_outA), (1, a2a_outB)):
                    for k in range(KT):
                        o_t = ots_pool.tile(
                            [128, RPC // 2], bf16, name=f"oTs_{k}", tag=f"ot{k}"
                        )
                        nc.sync.dma_start(
                            out=o_t[:], in_=buf_out[k * 128 : (k + 1) * 128, :]
                        )
                        last = k == KT - 1
                        for m in range(8):
                            nc.tensor.matmul(
                                pouts[m][:],
                                lhsT=wo_sb[k][:, m * 128 : (m + 1) * 128],
                                rhs=o_t[:],
                                start=(k == 0),
                                stop=last,
                            )
                            if last:
                                o_sb = out_pool.tile(
                                    [128, 256], bf16, name="o_sb", tag="o_sb"
                                )
                                nc.vector.tensor_scalar_add(
                                    out=o_sb[:],
                                    in0=pouts[m][:],
                                    scalar1=bias_o[:, m : m + 1],
                                )
                                nc.sync.dma_start(
                                    out=out[
                                        m * 128 : (m + 1) * 128,
                                        half * 256 : (half + 1) * 256,
                                    ],
                                    in_=o_sb[:],
                                )

    nc.compile()
    return nc


def _get_graph():
    if "nc" not in _CACHED:
        _CACHED["nc"] = _build_graph()
    return _CACHED["nc"]


def _make_in_maps(x, wqkv, bqkv, wo, bo):
    bf = ml_dtypes.bfloat16
    x2 = np.asarray(x, dtype=np.float32).reshape(R, DIM)
    xt = np.ascontiguousarray(x2.T.astype(bf))  # [dim, b*s] bf16
    wqkv = np.asarray(wqkv, dtype=np.float32)
    bqkv = np.asarray(bqkv, dtype=np.float32)
    wo16 = np.ascontiguousarray(np.asarray(wo, dtype=np.float32).astype(bf))
    bo_f = np.ascontiguousarray(np.asarray(bo, dtype=np.float32).reshape(8, 128))

    in_maps = []
    for c in range(NCORES):
        w_s = np.ascontiguousarray(
            np.concatenate(
                [
                    wqkv[:, c * FPC : (c + 1) * FPC],
                    wqkv[:, DIM + c * FPC : DIM + (c + 1) * FPC],
                    wqkv[:, 2 * DIM + c * FPC : 2 * DIM + (c + 1) * FPC],
                ],
                axis=1,
            ).astype(bf)
        )
        b_s = np.ascontiguousarray(
            np.stack(
                [
                    bqkv[c * FPC : (c + 1) * FPC],
                    bqkv[DIM + c * FPC : DIM + (c + 1) * FPC],
                    bqkv[2 * DIM + c * FPC : 2 * DIM + (c + 1) * FPC],
                ],
                axis=0,
            )
        )
        in_maps.append({"xt": xt, "wqkv": w_s, "bqkv": b_s, "wo": wo16, "bo": bo_f})
    return in_maps


def kernel(x, wqkv, bqkv, wo, bo):
    from concourse.bass_utils import run_bass_kernel_spmd

    nc = _get_graph()
    in_maps = _make_in_maps(x, wqkv, bqkv, wo, bo)
    res = run_bass_kernel_spmd(nc, in_maps, core_ids=list(range(NCORES)))
    outs = [res.results[c]["out"] for c in range(NCORES)]  # each [1024, 512]
    full = np.concatenate([o.T for o in outs], axis=0)  # [4096, 1024]
    return np.ascontiguousarray(full.reshape(B, S, DIM)).astype(np.float32)


# revision 33
# speedup vs baseline: 1.0104x; 1.0104x over previous
"""Distributed multi-head attention kernel for 8 TRN2 NeuronCores.

Sharding: tensor-parallel over heads (2 heads/core). Per core: qkv projection
for its 128 features, attention for its 2 heads, AllToAll exchange, then
row-parallel output projection (each core produces a transposed 512-row slice
of the final output); host reassembles.

Structure:
- V is projected directly in [rows, feat] layout (stationary = x k-tile,
  M = 128 rows): no PE transposes anywhere.
- PV uses column-split tile_position packing: both heads' PV matmuls run
  concurrently in column halves of the PE array (one 512-cycle pass per key
  tile instead of two M=65 passes).
- Softmax denominators come from a bf16 pair/quad/hex DVE reduction tree over
  the exp tiles plus 4 all-ones matmuls per block; reciprocals via the DVE
  reciprocal_approx_fast custom op (the scalar engine runs nothing but the
  128 exp tiles); normalization is two plain DVE multiplies whose recip
  operand layout matches po's partition split.
- The attention phase is ACT(exp)-bound, so everything else is threaded
  through its PE slack: the emission is software-pipelined (QK/exp of tile
  kt runs SHIFT tiles ahead of PV), the previous block's denominator/
  normalize work is spread over the next block's first tiles, and the whole
  remaining projection work (q for later blocks, all of batch 1) is emitted
  as fine-grained units popped between key tiles with deadline ordering.
- The exchange is split into two half-query AllToAlls so the first half's
  output projection overlaps the second collective (which pays no extra
  cross-core skew: the first collective already synced the cores); the
  output projection is k-outer so it starts as soon as the first received
  k-tile lands, with bias+store pipelined into the last k-slice.

Compute in bf16 on the PE array (f32 PSUM accumulation, f32 softmax
denominators/normalization). The host pre-transposes x to [dim, b*s] and
pre-casts x/wqkv/wo to bf16 as part of sharding/layout prep.
"""

import sys

sys.path.insert(0, "/opt/trn_rl_repo")

import ml_dtypes
import numpy as np

# Problem constants (hardcoded per harness contract)
B = 2
S = 2048
DIM = 1024
N_HEAD = 16
HD = 64  # head dim
SCALE = HD ** (-0.5)
R = B * S  # 4096 flattened rows
NCORES = 8
HPC = N_HEAD // NCORES  # 2 heads per core
FPC = HPC * HD  # 128 features per core
RPC = R // NCORES  # 512 rows per core (output row slice)

KT = DIM // 128  # 8 k-tiles over the model dim
NKT = S // 128  # 16 key tiles per sequence
NQB = S // 512  # 4 query blocks per sequence
SHIFT = 3  # PV pipeline lag behind QK/exp

_CACHED = {}


def _build_graph():
    import concourse.mybir as mybir
    import concourse.tile as tile
    from concourse import bacc

    nc = bacc.Bacc(
        "TRN2",
        target_bir_lowering=False,
        debug=False,
        num_devices=NCORES,
    )
    return _build_body(nc, mybir, tile)


def _build_body(nc, mybir, tile):
    f32 = mybir.dt.float32
    bf16 = mybir.dt.bfloat16
    EXP = mybir.ActivationFunctionType.Exp

    xt = nc.dram_tensor("xt", [DIM, R], bf16, kind="ExternalInput").ap()
    wqkv = nc.dram_tensor("wqkv", [DIM, 3 * FPC], bf16, kind="ExternalInput").ap()
    bqkv = nc.dram_tensor("bqkv", [3, FPC], f32, kind="ExternalInput").ap()
    wo = nc.dram_tensor("wo", [DIM, DIM], bf16, kind="ExternalInput").ap()
    bo = nc.dram_tensor("bo", [8, 128], f32, kind="ExternalInput").ap()
    out = nc.dram_tensor("out", [DIM, RPC], bf16, kind="ExternalOutput").ap()

    with tile.TileContext(nc) as tc:
        with (
            tc.tile_pool(name="glob", bufs=1) as glob,
            tc.tile_pool(name="dram", bufs=1, space="DRAM") as dram_pool,
        ):
            # ---------------- persistent tiles -------------------------
            ones128 = glob.tile([128, 128], bf16)
            nc.vector.memset(ones128[:], 1.0)
            bias_qkv = glob.tile([128, 2], f32)  # q, k per-partition biases
            vbias = glob.tile([128, 128], f32)  # v bias along free dim
            bias_o = glob.tile([128, 8], f32)
            qT = glob.tile([128, R], bf16)
            kT = glob.tile([128, R], bf16)
            v_nat = glob.tile([128, R], bf16)  # [keys, 2h*64d] per 128-chunk

            warm_in = dram_pool.tile([NCORES, 16], bf16, name="warm_in")
            warm_out = dram_pool.tile([NCORES, 16], bf16, name="warm_out")
            a2a_inA = dram_pool.tile([DIM, RPC // 2], bf16, name="a2a_inA")
            a2a_outA = dram_pool.tile([DIM, RPC // 2], bf16, name="a2a_outA")
            a2a_inB = dram_pool.tile([DIM, RPC // 2], bf16, name="a2a_inB")
            a2a_outB = dram_pool.tile([DIM, RPC // 2], bf16, name="a2a_outB")

            # ---------------- phase 0: weight/bias DMAs, warm a2a ------
            wqkv_sb = []
            for k in range(KT):
                w_t = glob.tile([128, 3 * FPC], bf16, name=f"w_{k}")
                nc.gpsimd.dma_start(out=w_t[:], in_=wqkv[k * 128 : (k + 1) * 128, :])
                wqkv_sb.append(w_t)
            for m in range(2):
                nc.gpsimd.dma_start(
                    out=bias_qkv[:, m : m + 1], in_=bqkv[m : m + 1, :]
                )
            nc.gpsimd.dma_start(
                out=vbias[:], in_=bqkv[2:3, :].to_broadcast((128, 128))
            )
            for m in range(8):
                nc.gpsimd.dma_start(out=bias_o[:, m : m + 1], in_=bo[m : m + 1, :])

            warm_sb = glob.tile([1, 16], bf16)
            nc.vector.memset(warm_sb[:], 1.0)
            nc.gpsimd.dma_start(out=warm_in[0:1, :], in_=warm_sb[0:1, :])
            nc.gpsimd.dma_start(
                out=warm_in[1:NCORES, :],
                in_=warm_in[0:1, :].to_broadcast((NCORES - 1, 16)),
            )
            nc.gpsimd.collective_compute(
                "AllToAll",
                mybir.AluOpType.bypass,
                replica_groups=[list(range(NCORES))],
                ins=[warm_in[:].opt()],
                outs=[warm_out[:].opt()],
            )
            wo_sb = []
            for k in range(KT):
                w_t = glob.tile([128, DIM], bf16, name=f"wo_{k}")
                nc.gpsimd.dma_start(out=w_t[:], in_=wo[k * 128 : (k + 1) * 128, :])
                wo_sb.append(w_t)

            with tc.tile_pool(name="xTp", bufs=2) as xT_pool:

                def dma_group(g, eng=None, split=False):
                    """DMA one 1024-row group of xt; returns the 8 k-tiles."""
                    eng = eng or nc.sync
                    xg = []
                    for k in range(KT):
                        t = xT_pool.tile(
                            [128, 1024], bf16, name=f"xT_{k}", tag=f"xT{k}"
                        )
                        e = nc.gpsimd if (split and k % 2) else eng
                        e.dma_start(
                            out=t[:],
                            in_=xt[
                                k * 128 : (k + 1) * 128, g * 1024 : (g + 1) * 1024
                            ],
                        )
                        xg.append(t)
                    return xg

                def qk_mms(pp, xg, m, h, ks):
                    for k in ks:
                        nc.tensor.matmul(
                            pp[:],
                            lhsT=wqkv_sb[k][:, m * 128 : (m + 1) * 128],
                            rhs=xg[k][:, h * 512 : (h + 1) * 512],
                            start=(k == 0),
                            stop=(k == KT - 1),
                        )

                def qk_bias(pp, g, m, h):
                    col0 = g * 1024 + h * 512
                    dst = qT if m == 0 else kT
                    nc.vector.tensor_scalar_add(
                        out=dst[:, col0 : col0 + 512],
                        in0=pp[:],
                        scalar1=bias_qkv[:, m : m + 1],
                    )

                def v_mms(vd, xg, c, ks):
                    for k in ks:
                        nc.tensor.matmul(
                            vd[:],
                            lhsT=xg[k][:, c * 128 : (c + 1) * 128],
                            rhs=wqkv_sb[k][:, 256:384],
                            start=(k == 0),
                            stop=(k == KT - 1),
                        )

                def v_bias(vd, g, c):
                    chunk = g * 8 + c
                    nc.vector.tensor_add(
                        out=v_nat[:, chunk * 128 : (chunk + 1) * 128],
                        in0=vd[:],
                        in1=vbias[:],
                    )

                # -------- phase 1 prefix: just enough to start block 0 --
                xgs = {}
                with (
                    tc.tile_pool(name="pp1", bufs=2, space="PSUM") as pp1_pool,
                    tc.tile_pool(name="vd1", bufs=1, space="PSUM") as vd1_pool,
                ):
                    xgs[0] = dma_group(0)
                    xgs[1] = dma_group(1)
                    # mini k-round: key tile 0 only, so the first QK/exp can
                    # issue several microseconds before the full rounds land
                    ppm = pp1_pool.tile([128, 128], f32, name="ppm", tag="pp")
                    for k in range(KT):
                        nc.tensor.matmul(
                            ppm[:],
                            lhsT=wqkv_sb[k][:, 128:256],
                            rhs=xgs[0][k][:, 0:128],
                            start=(k == 0),
                            stop=(k == KT - 1),
                        )
                    nc.vector.tensor_scalar_add(
                        out=kT[:, 0:128], in0=ppm[:], scalar1=bias_qkv[:, 1:2]
                    )
                    pp = pp1_pool.tile([128, 512], f32, name="pp", tag="pp")
                    qk_mms(pp, xgs[0], 0, 0, range(KT))  # q, rows 0-511
                    qk_bias(pp, 0, 0, 0)
                    pp = pp1_pool.tile([128, 512], f32, name="pp", tag="pp")
                    qk_mms(pp, xgs[0], 1, 0, range(KT))  # k, rows 0-511
                    qk_bias(pp, 0, 1, 0)
                    vd = vd1_pool.tile([128, 128], f32, name="vd", tag="vd")
                    v_mms(vd, xgs[0], 0, range(KT))  # v chunk 0
                    v_bias(vd, 0, 0)

                # -------- phase 2: attention + interleaved projection --
                with (
                    tc.tile_pool(name="pstp", bufs=2, space="PSUM") as pst_pool,
                    tc.tile_pool(name="pop", bufs=1, space="PSUM") as po_pool,
                    tc.tile_pool(name="denp", bufs=1, space="PSUM") as den_pool,
                    tc.tile_pool(name="pp2", bufs=1, space="PSUM") as pp2_pool,
                    tc.tile_pool(name="ptp", bufs=12) as pt_pool,
                    tc.tile_pool(name="pairp", bufs=2) as pair_pool,
                    tc.tile_pool(name="quadp", bufs=2) as quad_pool,
                    tc.tile_pool(name="hexp", bufs=2) as hex_pool,
                    tc.tile_pool(name="recipp", bufs=2) as recip_pool,
                    tc.tile_pool(name="oTsp", bufs=2) as oTs_pool,
                ):
                    st = {"pp": None, "vd": None, "pending": None}

                    # ---- deferred projection units (deadline-ordered) --
                    def u_dma(g):
                        return lambda: xgs.__setitem__(g, dma_group(g, nc.gpsimd))

                    def u_round_start(g, m, h, pool):
                        def f():
                            st["pp"] = pool.tile(
                                [128, 512], f32, name="pp", tag="pp"
                            )
                            qk_mms(st["pp"], xgs[g], m, h, range(2))

                        return f

                    def u_round_mid(g, m, h, ks):
                        return lambda: qk_mms(st["pp"], xgs[g], m, h, ks)

                    def u_round_end(g, m, h):
                        def f():
                            qk_mms(st["pp"], xgs[g], m, h, range(6, 8))
                            qk_bias(st["pp"], g, m, h)

                        return f

                    def round_units(g, m, h, pool):
                        return [
                            u_round_start(g, m, h, pool),
                            u_round_mid(g, m, h, range(2, 4)),
                            u_round_mid(g, m, h, range(4, 6)),
                            u_round_end(g, m, h),
                        ]

                    def u_v_a(g, c, pool):
                        def f():
                            st["vd"] = pool.tile(
                                [128, 128], f32, name="vd", tag="pp"
                            )
                            v_mms(st["vd"], xgs[g], c, range(4))

                        return f

                    def u_v_b(g, c):
                        def f():
                            v_mms(st["vd"], xgs[g], c, range(4, 8))
                            v_bias(st["vd"], g, c)

                        return f

                    def v_units(g, c, pool):
                        return [u_v_a(g, c, pool), u_v_b(g, c)]

                    p2 = pp2_pool
                    units = []
                    # batch-0 remainder, deadline-interleaved for block 0
                    # (3 pops/kt): v chunk c is needed by PV at kt c+SHIFT;
                    # k-round (g,h) covers key tiles g*8+h*4 .. +3.
                    units += v_units(0, 1, p2) + v_units(0, 2, p2)
                    units += round_units(0, 1, 1, p2)  # k rows 512-1023
                    units += v_units(0, 3, p2) + v_units(0, 4, p2)
                    units += round_units(1, 1, 0, p2)  # k rows 1024-1535
                    units += v_units(0, 5, p2) + v_units(0, 6, p2)
                    units += round_units(1, 1, 1, p2)  # k rows 1536-2047
                    units += v_units(0, 7, p2)
                    for c in range(8):
                        units += v_units(1, c, p2)  # v rows 1024-2047
                    units += round_units(0, 0, 1, p2)  # q for block 1
                    units += round_units(1, 0, 0, p2)  # q for block 2
                    units += round_units(1, 0, 1, p2)  # q for block 3
                    # batch 1: k and v (needed by block 4), q(g2,h0) too
                    units += [u_dma(2)]
                    units += round_units(2, 1, 0, p2) + round_units(2, 1, 1, p2)
                    for c in range(4):
                        units += v_units(2, c, p2)
                    units += [u_dma(3)]
                    for c in range(4, 8):
                        units += v_units(2, c, p2)
                    units += round_units(3, 1, 0, p2) + round_units(3, 1, 1, p2)
                    for c in range(8):
                        units += v_units(3, c, p2)
                    units += round_units(2, 0, 0, p2)  # q for block 4
                    # popped during blocks 4-6:
                    late_units = (
                        round_units(2, 0, 1, p2)  # q block 5
                        + round_units(3, 0, 0, p2)  # q block 6
                        + round_units(3, 0, 1, p2)  # q block 7
                    )
                    units.reverse()
                    late_units.reverse()

                    def emit_pv(blk, kt, pts, po, tree):
                        b = blk // NQB
                        off = (b * NKT + kt) * 128
                        pt = pts[kt]
                        nc.tensor.matmul(
                            po[0:64, :],
                            lhsT=v_nat[:, off : off + 64],
                            rhs=pt[:, 0:512],
                            start=(kt == 0),
                            stop=(kt == NKT - 1),
                            tile_position=(0, 0),
                        )
                        nc.tensor.matmul(
                            po[64:128, :],
                            lhsT=v_nat[:, off + 64 : off + 128],
                            rhs=pt[:, 512:1024],
                            start=(kt == 0),
                            stop=(kt == NKT - 1),
                            tile_position=(0, 64),
                        )
                        # bf16 reduction tree toward the denominators
                        if kt % 2 == 1:
                            pr = pair_pool.tile(
                                [128, 1024], bf16, name="pair", tag="pair"
                            )
                            nc.vector.tensor_add(
                                out=pr[:], in0=pts[kt - 1][:], in1=pt[:]
                            )
                            tree["pair"].append(pr)
                        if kt % 4 == 3:
                            qd = quad_pool.tile(
                                [128, 1024], bf16, name="quad", tag="quad"
                            )
                            nc.vector.tensor_add(
                                out=qd[:],
                                in0=tree["pair"][-2][:],
                                in1=tree["pair"][-1][:],
                            )
                            tree["quad"].append(qd)
                        if kt % 8 == 7:
                            hx = hex_pool.tile(
                                [128, 1024], bf16, name="hex", tag="hex"
                            )
                            nc.vector.tensor_add(
                                out=hx[:],
                                in0=tree["quad"][-2][:],
                                in1=tree["quad"][-1][:],
                            )
                            tree["hex"].append(hx)

                    def tail_a1(blk, pts, po, tree):
                        emit_pv(blk, NKT - 3, pts, po, tree)

                    def tail_a2(blk, pts, po, tree):
                        emit_pv(blk, NKT - 2, pts, po, tree)
                        emit_pv(blk, NKT - 1, pts, po, tree)

                    def tail_b(blk, pts, po, tree):
                        # denominator part 1: hex0 into both halves (start)
                        dn = den_pool.tile([128, 1024], f32, name="den", tag="den")
                        st["den"] = dn
                        hx0 = tree["hex"][0]
                        for half in range(2):
                            c0 = half * 512
                            nc.tensor.matmul(
                                dn[:, c0 : c0 + 512],
                                lhsT=ones128[:],
                                rhs=hx0[:, c0 : c0 + 512],
                                start=True,
                                stop=False,
                            )

                    def tail_c(blk, pts, po, tree):
                        dn = st["den"]
                        hx1 = tree["hex"][1]
                        for half in range(2):
                            c0 = half * 512
                            nc.tensor.matmul(
                                dn[:, c0 : c0 + 512],
                                lhsT=ones128[:],
                                rhs=hx1[:, c0 : c0 + 512],
                                start=False,
                                stop=True,
                            )
                        recip = recip_pool.tile(
                            [128, 1024], f32, name="recip", tag="rc"
                        )
                        nc.vector.reciprocal_approx_fast(out=recip[:], in_=dn[:])
                        oTs = oTs_pool.tile([128, 512], bf16, name="oTs", tag="oTs")
                        nc.vector.tensor_mul(
                            out=oTs[0:64, :],
                            in0=po[0:64, :],
                            in1=recip[0:64, 0:512],
                        )
                        nc.vector.tensor_mul(
                            out=oTs[64:128, :],
                            in0=po[64:128, :],
                            in1=recip[64:128, 512:1024],
                        )
                        nc.sync.dma_start(
                            out=a2a_inA[blk * 128 : (blk + 1) * 128, :],
                            in_=oTs[:, 0 : RPC // 2],
                        )
                        nc.sync.dma_start(
                            out=a2a_inB[blk * 128 : (blk + 1) * 128, :],
                            in_=oTs[:, RPC // 2 : RPC],
                        )

                    for b in range(B):
                        for qb in range(NQB):
                            blk = b * NQB + qb
                            q0 = b * S + qb * 512
                            pts = []
                            tree = {"pair": [], "quad": [], "hex": []}
                            po = None
                            for kt in range(NKT):
                                k0 = b * S + kt * 128
                                pst = pst_pool.tile(
                                    [128, 1024], f32, name="pst", tag="st"
                                )
                                for hh in range(HPC):
                                    nc.tensor.matmul(
                                        pst[:, hh * 512 : (hh + 1) * 512],
                                        lhsT=kT[
                                            hh * 64 : (hh + 1) * 64, k0 : k0 + 128
                                        ],
                                        rhs=qT[
                                            hh * 64 : (hh + 1) * 64, q0 : q0 + 512
                                        ],
                                        start=True,
                                        stop=True,
                                        tile_position=(hh * 64, 0),
                                    )
                                pt = pt_pool.tile(
                                    [128, 1024], bf16, name="ptile", tag="pt"
                                )
                                nc.scalar.activation(
                                    pt[:], pst[:], EXP, scale=SCALE
                                )
                                pts.append(pt)
                                pend = st["pending"]
                                if kt == 0 and pend:
                                    tail_a1(*pend)
                                elif kt == 1 and pend:
                                    tail_a2(*pend)
                                elif kt == 2 and pend:
                                    tail_b(*pend)
                                elif kt == 3 and pend:
                                    tail_c(*pend)
                                    st["pending"] = None
                                if blk == 7 and kt == 13:
                                    # last block: its hex0 is ready (kt10) so
                                    # the first denominator pair can run now,
                                    # shortening the pre-exchange tail chain
                                    dn = den_pool.tile(
                                        [128, 1024], f32, name="den", tag="den"
                                    )
                                    st["den"] = dn
                                    hx0 = tree["hex"][0]
                                    for half in range(2):
                                        c0 = half * 512
                                        nc.tensor.matmul(
                                            dn[:, c0 : c0 + 512],
                                            lhsT=ones128[:],
                                            rhs=hx0[:, c0 : c0 + 512],
                                            start=True,
                                            stop=False,
                                        )
                                if kt == SHIFT:
                                    po = po_pool.tile(
                                        [128, 512], f32, name="po", tag="po"
                                    )
                                if kt >= SHIFT:
                                    emit_pv(blk, kt - SHIFT, pts, po, tree)
                                # deadline-paced unit pops, kept away from the
                                # block-boundary key-tiles that carry the
                                # previous block's denominator/normalize work
                                npop = 0
                                if blk == 0:
                                    npop = 3 if kt < 14 else 2
                                elif blk < 4:
                                    npop = 0 if kt < 4 else (2 if kt < 14 else 1)
                                elif blk < 7:
                                    npop = 1 if kt in (5, 7, 9, 11) else 0
                                for _ in range(npop):
                                    if blk < 4 and units:
                                        units.pop()()
                                    elif late_units:
                                        late_units.pop()()
                            st["pending"] = (blk, pts, po, tree)
                    # flush the last block (denA already ran at kt13)
                    tail_a1(*st["pending"])
                    tail_a2(*st["pending"])
                    tail_c(*st["pending"])
                    st["pending"] = None
                    while units:
                        units.pop()()
                    while late_units:
                        late_units.pop()()
                    for buf_in, buf_out in ((a2a_inA, a2a_outA), (a2a_inB, a2a_outB)):
                        nc.gpsimd.collective_compute(
                            "AllToAll",
                            mybir.AluOpType.bypass,
                            replica_groups=[list(range(NCORES))],
                            ins=[buf_in[:].opt()],
                            outs=[buf_out[:].opt()],
                        )

            # ---------------- phase 3: output projection ---------------
            with (
                tc.tile_pool(name="ots", bufs=1) as ots_pool,
                tc.tile_pool(name="psout", bufs=1, space="PSUM") as ps_out,
                tc.tile_pool(name="outt", bufs=2) as out_pool,
            ):
                pouts = [
                    ps_out.tile([128, 256], f32, name=f"pout{m}", tag=f"po{m}")
                    for m in range(8)
                ]
                for half, buf_out in ((0, a2a_outA), (1, a2a_outB)):
                    for k in range(KT):
                        o_t = ots_pool.tile(
                            [128, RPC // 2], bf16, name=f"oTs_{k}", tag=f"ot{k}"
                        )
                        nc.sync.dma_start(
                            out=o_t[:], in_=buf_out[k * 128 : (k + 1) * 128, :]
                        )
                        last = k == KT - 1
                        for m in range(8):
                            nc.tensor.matmul(
                                pouts[m][:],
                                lhsT=wo_sb[k][:, m * 128 : (m + 1) * 128],
                                rhs=o_t[:],
                                start=(k == 0),
                                stop=last,
                            )
                            if last:
                                o_sb = out_pool.tile(
                                    [128, 256], bf16, name="o_sb", tag="o_sb"
                                )
                                nc.vector.tensor_scalar_add(
                                    out=o_sb[:],
                                    in0=pouts[m][:],
                                    scalar1=bias_o[:, m : m + 1],
                                )
                                nc.sync.dma_start(
                                    out=out[
                                        m * 128 : (m + 1) * 128,
                                        half * 256 : (half + 1) * 256,
                                    ],
                                    in_=o_sb[:],
                                )

    nc.compile()
    return nc


def _get_graph():
    if "nc" not in _CACHED:
        _CACHED["nc"] = _build_graph()
    return _CACHED["nc"]


def _make_in_maps(x, wqkv, bqkv, wo, bo):
    bf = ml_dtypes.bfloat16
    x2 = np.asarray(x, dtype=np.float32).reshape(R, DIM)
    xt = np.ascontiguousarray(x2.T.astype(bf))  # [dim, b*s] bf16
    wqkv = np.asarray(wqkv, dtype=np.float32)
    bqkv = np.asarray(bqkv, dtype=np.float32)
    wo16 = np.ascontiguousarray(np.asarray(wo, dtype=np.float32).astype(bf))
    bo_f = np.ascontiguousarray(np.asarray(bo, dtype=np.float32).reshape(8, 128))

    in_maps = []
    for c in range(NCORES):
        w_s = np.ascontiguousarray(
            np.concatenate(
                [
                    wqkv[:, c * FPC : (c + 1) * FPC],
                    wqkv[:, DIM + c * FPC : DIM + (c + 1) * FPC],
                    wqkv[:, 2 * DIM + c * FPC : 2 * DIM + (c + 1) * FPC],
                ],
                axis=1,
            ).astype(bf)
        )
        b_s = np.ascontiguousarray(
            np.stack(
                [
                    bqkv[c * FPC : (c + 1) * FPC],
                    bqkv[DIM + c * FPC : DIM + (c + 1) * FPC],
                    bqkv[2 * DIM + c * FPC : 2 * DIM + (c + 1) * FPC],
                ],
                axis=0,
            )
        )
        in_maps.append({"xt": xt, "wqkv": w_s, "bqkv": b_s, "wo": wo16, "bo": bo_f})
    return in_maps


def kernel(x, wqkv, bqkv, wo, bo):
    from concourse.bass_utils import run_bass_kernel_spmd

    nc = _get_graph()
    in_maps = _make_in_maps(x, wqkv, bqkv, wo, bo)
    res = run_bass_kernel_spmd(nc, in_maps, core_ids=list(range(NCORES)))
    outs = [res.results[c]["out"] for c in range(NCORES)]  # each [1024, 512]
    full = np.concatenate([o.T for o in outs], axis=0)  # [4096, 1024]
    return np.ascontiguousarray(full.reshape(B, S, DIM)).astype(np.float32)


# revision 34
# speedup vs baseline: 1.0727x; 1.0617x over previous
"""Distributed multi-head attention kernel for 8 TRN2 NeuronCores.

Sharding: tensor-parallel over heads (2 heads/core). Per core: qkv projection
for its 128 features, attention for its 2 heads, AllToAll exchange, then
row-parallel output projection (each core produces a transposed 512-row slice
of the final output); host reassembles.

Structure:
- V is projected directly in [rows, feat] layout (stationary = x k-tile,
  M = 128 rows): no PE transposes anywhere.
- PV uses column-split tile_position packing: both heads' PV matmuls run
  concurrently in column halves of the PE array (one 512-cycle pass per key
  tile instead of two M=65 passes).
- Softmax denominators come from a bf16 pair/quad/hex DVE reduction tree over
  the exp tiles plus 4 all-ones matmuls per block; reciprocals via the DVE
  reciprocal_approx_fast custom op (the scalar engine runs nothing but the
  128 exp tiles); normalization is two plain DVE multiplies whose recip
  operand layout matches po's partition split.
- The attention phase is ACT(exp)-bound, so everything else is threaded
  through its PE slack: the emission is software-pipelined (QK/exp of tile
  kt runs SHIFT tiles ahead of PV), the previous block's denominator/
  normalize work is spread over the next block's first tiles, and the whole
  remaining projection work (q for later blocks, all of batch 1) is emitted
  as fine-grained units popped between key tiles with deadline ordering.
- The exchange is split into two half-query AllToAlls so the first half's
  output projection overlaps the second collective (which pays no extra
  cross-core skew: the first collective already synced the cores); the
  output projection is k-outer so it starts as soon as the first received
  k-tile lands, with bias+store pipelined into the last k-slice.

Compute in bf16 on the PE array (f32 PSUM accumulation, f32 softmax
denominators/normalization). The host pre-transposes x to [dim, b*s] and
pre-casts x/wqkv/wo to bf16 as part of sharding/layout prep.
"""

import sys

sys.path.insert(0, "/opt/trn_rl_repo")

import ml_dtypes
import numpy as np

# Problem constants (hardcoded per harness contract)
B = 2
S = 2048
DIM = 1024
N_HEAD = 16
HD = 64  # head dim
SCALE = HD ** (-0.5)
R = B * S  # 4096 flattened rows
NCORES = 8
HPC = N_HEAD // NCORES  # 2 heads per core
FPC = HPC * HD  # 128 features per core
RPC = R // NCORES  # 512 rows per core (output row slice)

KT = DIM // 128  # 8 k-tiles over the model dim
NKT = S // 128  # 16 key tiles per sequence
NQB = S // 512  # 4 query blocks per sequence
SHIFT = 3  # PV pipeline lag behind QK/exp

_CACHED = {}


def _build_graph():
    import concourse.mybir as mybir
    import concourse.tile as tile
    from concourse import bacc

    nc = bacc.Bacc(
        "TRN2",
        target_bir_lowering=False,
        debug=False,
        num_devices=NCORES,
    )
    return _build_body(nc, mybir, tile)


def _build_body(nc, mybir, tile):
    f32 = mybir.dt.float32
    bf16 = mybir.dt.bfloat16
    EXP = mybir.ActivationFunctionType.Exp

    xt = nc.dram_tensor("xt", [DIM, R], bf16, kind="ExternalInput").ap()
    wqkv = nc.dram_tensor("wqkv", [DIM, 3 * FPC], bf16, kind="ExternalInput").ap()
    bqkv = nc.dram_tensor("bqkv", [3, FPC], f32, kind="ExternalInput").ap()
    wo = nc.dram_tensor("wo", [DIM, DIM], bf16, kind="ExternalInput").ap()
    bo = nc.dram_tensor("bo", [8, 128], f32, kind="ExternalInput").ap()
    out = nc.dram_tensor("out", [DIM, RPC], bf16, kind="ExternalOutput").ap()

    with tile.TileContext(nc) as tc:
        with (
            tc.tile_pool(name="glob", bufs=1) as glob,
            tc.tile_pool(name="dram", bufs=1, space="DRAM") as dram_pool,
        ):
            # ---------------- persistent tiles -------------------------
            ones128 = glob.tile([128, 128], bf16)
            nc.vector.memset(ones128[:], 1.0)
            bias_qkv = glob.tile([128, 2], f32)  # q, k per-partition biases
            vbias = glob.tile([128, 128], f32)  # v bias along free dim
            bias_o = glob.tile([128, 8], f32)
            qT = glob.tile([128, R], bf16)
            kT = glob.tile([128, R], bf16)
            v_nat = glob.tile([128, R], bf16)  # [keys, 2h*64d] per 128-chunk

            warm_in = dram_pool.tile([NCORES, 16], bf16, name="warm_in")
            warm_out = dram_pool.tile([NCORES, 16], bf16, name="warm_out")
            a2a_inA = dram_pool.tile([DIM, RPC // 2], bf16, name="a2a_inA")
            a2a_outA = dram_pool.tile([DIM, RPC // 2], bf16, name="a2a_outA")
            a2a_inB = dram_pool.tile([DIM, RPC // 2], bf16, name="a2a_inB")
            a2a_outB = dram_pool.tile([DIM, RPC // 2], bf16, name="a2a_outB")

            # ---------------- phase 0: weight/bias DMAs, warm a2a ------
            wqkv_sb = []
            for k in range(KT):
                w_t = glob.tile([128, 3 * FPC], bf16, name=f"w_{k}")
                nc.gpsimd.dma_start(out=w_t[:], in_=wqkv[k * 128 : (k + 1) * 128, :])
                wqkv_sb.append(w_t)
            for m in range(2):
                nc.gpsimd.dma_start(
                    out=bias_qkv[:, m : m + 1], in_=bqkv[m : m + 1, :]
                )
            nc.gpsimd.dma_start(
                out=vbias[:], in_=bqkv[2:3, :].to_broadcast((128, 128))
            )
            for m in range(8):
                nc.gpsimd.dma_start(out=bias_o[:, m : m + 1], in_=bo[m : m + 1, :])

            warm_sb = glob.tile([1, 16], bf16)
            nc.vector.memset(warm_sb[:], 1.0)
            # touch Exp immediately so the ~1.3us ACT table load happens
            # during the initial DMA wait, not before the first real exp
            warm_act = glob.tile([1, 16], f32)
            nc.scalar.activation(warm_act[:], warm_sb[:], EXP)
            nc.gpsimd.dma_start(out=warm_in[0:1, :], in_=warm_sb[0:1, :])
            nc.gpsimd.dma_start(
                out=warm_in[1:NCORES, :],
                in_=warm_in[0:1, :].to_broadcast((NCORES - 1, 16)),
            )
            nc.gpsimd.collective_compute(
                "AllToAll",
                mybir.AluOpType.bypass,
                replica_groups=[list(range(NCORES))],
                ins=[warm_in[:].opt()],
                outs=[warm_out[:].opt()],
            )
            wo_sb = []
            for k in range(KT):
                w_t = glob.tile([128, DIM], bf16, name=f"wo_{k}")
                nc.gpsimd.dma_start(out=w_t[:], in_=wo[k * 128 : (k + 1) * 128, :])
                wo_sb.append(w_t)

            with tc.tile_pool(name="xTp", bufs=2) as xT_pool:

                def dma_group(g, eng=None, split=False):
                    """DMA one 1024-row group of xt; returns the 8 k-tiles."""
                    eng = eng or nc.sync
                    xg = []
                    for k in range(KT):
                        t = xT_pool.tile(
                            [128, 1024], bf16, name=f"xT_{k}", tag=f"xT{k}"
                        )
                        e = nc.gpsimd if (split and k % 2) else eng
                        e.dma_start(
                            out=t[:],
                            in_=xt[
                                k * 128 : (k + 1) * 128, g * 1024 : (g + 1) * 1024
                            ],
                        )
                        xg.append(t)
                    return xg

                def qk_mms(pp, xg, m, h, ks):
                    for k in ks:
                        nc.tensor.matmul(
                            pp[:],
                            lhsT=wqkv_sb[k][:, m * 128 : (m + 1) * 128],
                            rhs=xg[k][:, h * 512 : (h + 1) * 512],
                            start=(k == 0),
                            stop=(k == KT - 1),
                        )

                def qk_bias(pp, g, m, h):
                    col0 = g * 1024 + h * 512
                    dst = qT if m == 0 else kT
                    nc.vector.tensor_scalar_add(
                        out=dst[:, col0 : col0 + 512],
                        in0=pp[:],
                        scalar1=bias_qkv[:, m : m + 1],
                    )

                def v_mms(vd, xg, c, ks):
                    for k in ks:
                        nc.tensor.matmul(
                            vd[:],
                            lhsT=xg[k][:, c * 128 : (c + 1) * 128],
                            rhs=wqkv_sb[k][:, 256:384],
                            start=(k == 0),
                            stop=(k == KT - 1),
                        )

                def v_bias(vd, g, c):
                    chunk = g * 8 + c
                    nc.vector.tensor_add(
                        out=v_nat[:, chunk * 128 : (chunk + 1) * 128],
                        in0=vd[:],
                        in1=vbias[:],
                    )

                # -------- phase 1 prefix: just enough to start block 0 --
                xgs = {}
                with (
                    tc.tile_pool(name="pp1", bufs=2, space="PSUM") as pp1_pool,
                    tc.tile_pool(name="vd1", bufs=1, space="PSUM") as vd1_pool,
                ):
                    xgs[0] = dma_group(0)
                    xgs[1] = dma_group(1)
                    # mini k-round: key tile 0 only, so the first QK/exp can
                    # issue several microseconds before the full rounds land
                    ppm = pp1_pool.tile([128, 128], f32, name="ppm", tag="pp")
                    for k in range(KT):
                        nc.tensor.matmul(
                            ppm[:],
                            lhsT=wqkv_sb[k][:, 128:256],
                            rhs=xgs[0][k][:, 0:128],
                            start=(k == 0),
                            stop=(k == KT - 1),
                        )
                    nc.vector.tensor_scalar_add(
                        out=kT[:, 0:128], in0=ppm[:], scalar1=bias_qkv[:, 1:2]
                    )
                    pp = pp1_pool.tile([128, 512], f32, name="pp", tag="pp")
                    qk_mms(pp, xgs[0], 0, 0, range(KT))  # q, rows 0-511
                    qk_bias(pp, 0, 0, 0)
                    pp = pp1_pool.tile([128, 512], f32, name="pp", tag="pp")
                    qk_mms(pp, xgs[0], 1, 0, range(KT))  # k, rows 0-511
                    qk_bias(pp, 0, 1, 0)
                    vd = vd1_pool.tile([128, 128], f32, name="vd", tag="vd")
                    v_mms(vd, xgs[0], 0, range(KT))  # v chunk 0
                    v_bias(vd, 0, 0)

                # -------- phase 2: attention + interleaved projection --
                with (
                    tc.tile_pool(name="pstp", bufs=2, space="PSUM") as pst_pool,
                    tc.tile_pool(name="pop", bufs=1, space="PSUM") as po_pool,
                    tc.tile_pool(name="denp", bufs=1, space="PSUM") as den_pool,
                    tc.tile_pool(name="pp2", bufs=1, space="PSUM") as pp2_pool,
                    tc.tile_pool(name="ptp", bufs=12) as pt_pool,
                    tc.tile_pool(name="pairp", bufs=2) as pair_pool,
                    tc.tile_pool(name="quadp", bufs=2) as quad_pool,
                    tc.tile_pool(name="hexp", bufs=2) as hex_pool,
                    tc.tile_pool(name="recipp", bufs=2) as recip_pool,
                    tc.tile_pool(name="oTsp", bufs=2) as oTs_pool,
                ):
                    st = {"pp": None, "vd": None, "pending": None}

                    # ---- deferred projection units (deadline-ordered) --
                    def u_dma(g):
                        return lambda: xgs.__setitem__(g, dma_group(g, nc.gpsimd))

                    def u_round_start(g, m, h, pool):
                        def f():
                            st["pp"] = pool.tile(
                                [128, 512], f32, name="pp", tag="pp"
                            )
                            qk_mms(st["pp"], xgs[g], m, h, range(2))

                        return f

                    def u_round_mid(g, m, h, ks):
                        return lambda: qk_mms(st["pp"], xgs[g], m, h, ks)

                    def u_round_end(g, m, h):
                        def f():
                            qk_mms(st["pp"], xgs[g], m, h, range(6, 8))
                            qk_bias(st["pp"], g, m, h)

                        return f

                    def round_units(g, m, h, pool):
                        return [
                            u_round_start(g, m, h, pool),
                            u_round_mid(g, m, h, range(2, 4)),
                            u_round_mid(g, m, h, range(4, 6)),
                            u_round_end(g, m, h),
                        ]

                    def u_v_a(g, c, pool):
                        def f():
                            st["vd"] = pool.tile(
                                [128, 128], f32, name="vd", tag="pp"
                            )
                            v_mms(st["vd"], xgs[g], c, range(4))

                        return f

                    def u_v_b(g, c):
                        def f():
                            v_mms(st["vd"], xgs[g], c, range(4, 8))
                            v_bias(st["vd"], g, c)

                        return f

                    def v_units(g, c, pool):
                        return [u_v_a(g, c, pool), u_v_b(g, c)]

                    p2 = pp2_pool
                    units = []
                    # batch-0 remainder, deadline-interleaved for block 0
                    # (3 pops/kt): v chunk c is needed by PV at kt c+SHIFT;
                    # k-round (g,h) covers key tiles g*8+h*4 .. +3.
                    units += v_units(0, 1, p2) + v_units(0, 2, p2)
                    units += round_units(0, 1, 1, p2)  # k rows 512-1023
                    units += v_units(0, 3, p2) + v_units(0, 4, p2)
                    units += round_units(1, 1, 0, p2)  # k rows 1024-1535
                    units += v_units(0, 5, p2) + v_units(0, 6, p2)
                    units += round_units(1, 1, 1, p2)  # k rows 1536-2047
                    units += v_units(0, 7, p2)
                    for c in range(8):
                        units += v_units(1, c, p2)  # v rows 1024-2047
                    units += round_units(0, 0, 1, p2)  # q for block 1
                    units += round_units(1, 0, 0, p2)  # q for block 2
                    units += round_units(1, 0, 1, p2)  # q for block 3
                    # batch 1: k and v (needed by block 4), q(g2,h0) too
                    units += [u_dma(2)]
                    units += round_units(2, 1, 0, p2) + round_units(2, 1, 1, p2)
                    for c in range(4):
                        units += v_units(2, c, p2)
                    units += [u_dma(3)]
                    for c in range(4, 8):
                        units += v_units(2, c, p2)
                    units += round_units(3, 1, 0, p2) + round_units(3, 1, 1, p2)
                    for c in range(8):
                        units += v_units(3, c, p2)
                    units += round_units(2, 0, 0, p2)  # q for block 4
                    # popped during blocks 4-6:
                    late_units = (
                        round_units(2, 0, 1, p2)  # q block 5
                        + round_units(3, 0, 0, p2)  # q block 6
                        + round_units(3, 0, 1, p2)  # q block 7
                    )
                    units.reverse()
                    late_units.reverse()

                    def emit_pv(blk, kt, pts, po, tree):
                        b = blk // NQB
                        off = (b * NKT + kt) * 128
                        pt = pts[kt]
                        nc.tensor.matmul(
                            po[0:64, :],
                            lhsT=v_nat[:, off : off + 64],
                            rhs=pt[:, 0:512],
                            start=(kt == 0),
                            stop=(kt == NKT - 1),
                            tile_position=(0, 0),
                        )
                        nc.tensor.matmul(
                            po[64:128, :],
                            lhsT=v_nat[:, off + 64 : off + 128],
                            rhs=pt[:, 512:1024],
                            start=(kt == 0),
                            stop=(kt == NKT - 1),
                            tile_position=(0, 64),
                        )
                        # bf16 reduction tree toward the denominators
                        if kt % 2 == 1:
                            pr = pair_pool.tile(
                                [128, 1024], bf16, name="pair", tag="pair"
                            )
                            nc.vector.tensor_add(
                                out=pr[:], in0=pts[kt - 1][:], in1=pt[:]
                            )
                            tree["pair"].append(pr)
                        if kt % 4 == 3:
                            qd = quad_pool.tile(
                                [128, 1024], bf16, name="quad", tag="quad"
                            )
                            nc.vector.tensor_add(
                                out=qd[:],
                                in0=tree["pair"][-2][:],
                                in1=tree["pair"][-1][:],
                            )
                            tree["quad"].append(qd)
                        if kt % 8 == 7:
                            hx = hex_pool.tile(
                                [128, 1024], bf16, name="hex", tag="hex"
                            )
                            nc.vector.tensor_add(
                                out=hx[:],
                                in0=tree["quad"][-2][:],
                                in1=tree["quad"][-1][:],
                            )
                            tree["hex"].append(hx)

                    def tail_a1(blk, pts, po, tree):
                        emit_pv(blk, NKT - 3, pts, po, tree)

                    def tail_a2(blk, pts, po, tree):
                        emit_pv(blk, NKT - 2, pts, po, tree)
                        emit_pv(blk, NKT - 1, pts, po, tree)

                    def tail_b(blk, pts, po, tree):
                        # denominator part 1: hex0 into both halves (start)
                        dn = den_pool.tile([128, 1024], f32, name="den", tag="den")
                        st["den"] = dn
                        hx0 = tree["hex"][0]
                        for half in range(2):
                            c0 = half * 512
                            nc.tensor.matmul(
                                dn[:, c0 : c0 + 512],
                                lhsT=ones128[:],
                                rhs=hx0[:, c0 : c0 + 512],
                                start=True,
                                stop=False,
                            )

                    def tail_c(blk, pts, po, tree):
                        dn = st["den"]
                        hx1 = tree["hex"][1]
                        for half in range(2):
                            c0 = half * 512
                            nc.tensor.matmul(
                                dn[:, c0 : c0 + 512],
                                lhsT=ones128[:],
                                rhs=hx1[:, c0 : c0 + 512],
                                start=False,
                                stop=True,
                            )
                        recip = recip_pool.tile(
                            [128, 1024], f32, name="recip", tag="rc"
                        )
                        nc.vector.reciprocal_approx_fast(out=recip[:], in_=dn[:])
                        oTs = oTs_pool.tile([128, 512], bf16, name="oTs", tag="oTs")
                        nc.vector.tensor_mul(
                            out=oTs[0:64, :],
                            in0=po[0:64, :],
                            in1=recip[0:64, 0:512],
                        )
                        nc.vector.tensor_mul(
                            out=oTs[64:128, :],
                            in0=po[64:128, :],
                            in1=recip[64:128, 512:1024],
                        )
                        nc.sync.dma_start(
                            out=a2a_inA[blk * 128 : (blk + 1) * 128, :],
                            in_=oTs[:, 0 : RPC // 2],
                        )
                        nc.sync.dma_start(
                            out=a2a_inB[blk * 128 : (blk + 1) * 128, :],
                            in_=oTs[:, RPC // 2 : RPC],
                        )

                    for b in range(B):
                        for qb in range(NQB):
                            blk = b * NQB + qb
                            q0 = b * S + qb * 512
                            pts = []
                            tree = {"pair": [], "quad": [], "hex": []}
                            po = None
                            for kt in range(NKT):
                                k0 = b * S + kt * 128
                                pst = pst_pool.tile(
                                    [128, 1024], f32, name="pst", tag="st"
                                )
                                for hh in range(HPC):
                                    nc.tensor.matmul(
                                        pst[:, hh * 512 : (hh + 1) * 512],
                                        lhsT=kT[
                                            hh * 64 : (hh + 1) * 64, k0 : k0 + 128
                                        ],
                                        rhs=qT[
                                            hh * 64 : (hh + 1) * 64, q0 : q0 + 512
                                        ],
                                        start=True,
                                        stop=True,
                                        tile_position=(hh * 64, 0),
                                    )
                                pt = pt_pool.tile(
                                    [128, 1024], bf16, name="ptile", tag="pt"
                                )
                                nc.scalar.activation(
                                    pt[:], pst[:], EXP, scale=SCALE
                                )
                                pts.append(pt)
                                pend = st["pending"]
                                if kt == 0 and pend:
                                    tail_a1(*pend)
                                elif kt == 1 and pend:
                                    tail_a2(*pend)
                                elif kt == 2 and pend:
                                    tail_b(*pend)
                                elif kt == 3 and pend:
                                    tail_c(*pend)
                                    st["pending"] = None
                                if blk == 7 and kt == 13:
                                    # last block: its hex0 is ready (kt10) so
                                    # the first denominator pair can run now,
                                    # shortening the pre-exchange tail chain
                                    dn = den_pool.tile(
                                        [128, 1024], f32, name="den", tag="den"
                                    )
                                    st["den"] = dn
                                    hx0 = tree["hex"][0]
                                    for half in range(2):
                                        c0 = half * 512
                                        nc.tensor.matmul(
                                            dn[:, c0 : c0 + 512],
                                            lhsT=ones128[:],
                                            rhs=hx0[:, c0 : c0 + 512],
                                            start=True,
                                            stop=False,
                                        )
                                if kt == SHIFT:
                                    po = po_pool.tile(
                                        [128, 512], f32, name="po", tag="po"
                                    )
                                if kt >= SHIFT:
                                    emit_pv(blk, kt - SHIFT, pts, po, tree)
                                # deadline-paced unit pops, kept away from the
                                # block-boundary key-tiles that carry the
                                # previous block's denominator/normalize work
                                npop = 0
                                if blk == 0:
                                    npop = 3 if kt < 14 else 2
                                elif blk < 4:
                                    npop = 0 if kt < 4 else (2 if kt < 14 else 1)
                                elif blk < 7:
                                    npop = 1 if kt in (5, 7, 9, 11) else 0
                                for _ in range(npop):
                                    if blk < 4 and units:
                                        units.pop()()
                                    elif late_units:
                                        late_units.pop()()
                            st["pending"] = (blk, pts, po, tree)
                    # flush the last block (denA already ran at kt13)
                    tail_a1(*st["pending"])
                    tail_a2(*st["pending"])
                    tail_c(*st["pending"])
                    st["pending"] = None
                    while units:
                        units.pop()()
                    while late_units:
                        late_units.pop()()
                    for buf_in, buf_out in ((a2a_inA, a2a_outA), (a2a_inB, a2a_outB)):
                        nc.gpsimd.collective_compute(
                            "AllToAll",
                            mybir.AluOpType.bypass,
                            replica_groups=[list(range(NCORES))],
                            ins=[buf_in[:].opt()],
                            outs=[buf_out[:].opt()],
                        )

            # ---------------- phase 3: output projection ---------------
            with (
                tc.tile_pool(name="ots", bufs=1) as ots_pool,
                tc.tile_pool(name="psout", bufs=1, space="PSUM") as ps_out,
                tc.tile_pool(name="outt", bufs=2) as out_pool,
            ):
                pouts = [
                    ps_out.tile([128, 256], f32, name=f"pout{m}", tag=f"po{m}")
                    for m in range(8)
                ]
                for half, buf_out in ((0, a2a_outA), (1, a2a_outB)):
                    for k in range(KT):
                        o_t = ots_pool.tile(
                            [128, RPC // 2], bf16, name=f"oTs_{k}", tag=f"ot{k}"
                        )
                        nc.sync.dma_start(
                            out=o_t[:], in_=buf_out[k * 128 : (k + 1) * 128, :]
                        )
                        last = k == KT - 1
                        for m in range(8):
                            nc.tensor.matmul(
                                pouts[m][:],
                                lhsT=wo_sb[k][:, m * 128 : (m + 1) * 128],
                                rhs=o_t[:],
                                start=(k == 0),
                                stop=last,
                            )
                            if last:
                                o_sb = out_pool.tile(
                                    [128, 256], bf16, name="o_sb", tag="o_sb"
                                )
                                nc.vector.tensor_scalar_add(
                                    out=o_sb[:],
                                    in0=pouts[m][:],
                                    scalar1=bias_o[:, m : m + 1],
                                )
                                nc.sync.dma_start(
                                    out=out[
                                        m * 128 : (m + 1) * 128,
                                        half * 256 : (half + 1) * 256,
                                    ],
                                    in_=o_sb[:],
                                )

    nc.compile()
    return nc


def _get_graph():
    if "nc" not in _CACHED:
        _CACHED["nc"] = _build_graph()
    return _CACHED["nc"]


def _make_in_maps(x, wqkv, bqkv, wo, bo):
    bf = ml_dtypes.bfloat16
    x2 = np.asarray(x, dtype=np.float32).reshape(R, DIM)
    xt = np.ascontiguousarray(x2.T.astype(bf))  # [dim, b*s] bf16
    wqkv = np.asarray(wqkv, dtype=np.float32)
    bqkv = np.asarray(bqkv, dtype=np.float32)
    wo16 = np.ascontiguousarray(np.asarray(wo, dtype=np.float32).astype(bf))
    bo_f = np.ascontiguousarray(np.asarray(bo, dtype=np.float32).reshape(8, 128))

    in_maps = []
    for c in range(NCORES):
        w_s = np.ascontiguousarray(
            np.concatenate(
                [
                    wqkv[:, c * FPC : (c + 1) * FPC],
                    wqkv[:, DIM + c * FPC : DIM + (c + 1) * FPC],
                    wqkv[:, 2 * DIM + c * FPC : 2 * DIM + (c + 1) * FPC],
                ],
                axis=1,
            ).astype(bf)
        )
        b_s = np.ascontiguousarray(
            np.stack(
                [
                    bqkv[c * FPC : (c + 1) * FPC],
                    bqkv[DIM + c * FPC : DIM + (c + 1) * FPC],
                    bqkv[2 * DIM + c * FPC : 2 * DIM + (c + 1) * FPC],
                ],
                axis=0,
            )
        )
        in_maps.append({"xt": xt, "wqkv": w_s, "bqkv": b_s, "wo": wo16, "bo": bo_f})
    return in_maps


def kernel(x, wqkv, bqkv, wo, bo):
    from concourse.bass_utils import run_bass_kernel_spmd

    nc = _get_graph()
    in_maps = _make_in_maps(x, wqkv, bqkv, wo, bo)
    res = run_bass_kernel_spmd(nc, in_maps, core_ids=list(range(NCORES)))
    outs = [res.results[c]["out"] for c in range(NCORES)]  # each [1024, 512]
    full = np.concatenate([o.T for o in outs], axis=0)  # [4096, 1024]
    return np.ascontiguousarray(full.reshape(B, S, DIM)).astype(np.float32)


# revision 35
# speedup vs baseline: 1.0770x; 1.0040x over previous
"""Distributed multi-head attention kernel for 8 TRN2 NeuronCores.

Sharding: tensor-parallel over heads (2 heads/core). Per core: qkv projection
for its 128 features, attention for its 2 heads, AllToAll exchange, then
row-parallel output projection (each core produces a transposed 512-row slice
of the final output); host reassembles.

Structure:
- V is projected directly in [rows, feat] layout (stationary = x k-tile,
  M = 128 rows): no PE transposes anywhere.
- PV uses column-split tile_position packing: both heads' PV matmuls run
  concurrently in column halves of the PE array (one 512-cycle pass per key
  tile instead of two M=65 passes).
- Softmax denominators come from a bf16 pair/quad/hex DVE reduction tree over
  the exp tiles plus 4 all-ones matmuls per block; reciprocals via the DVE
  reciprocal_approx_fast custom op (the scalar engine runs nothing but the
  128 exp tiles); normalization is two plain DVE multiplies whose recip
  operand layout matches po's partition split.
- The attention phase is ACT(exp)-bound, so everything else is threaded
  through its PE slack: the emission is software-pipelined (QK/exp of tile
  kt runs SHIFT tiles ahead of PV), the previous block's denominator/
  normalize work is spread over the next block's first tiles, and the whole
  remaining projection work (q for later blocks, all of batch 1) is emitted
  as fine-grained units popped between key tiles with deadline ordering.
- The exchange is split into two half-query AllToAlls so the first half's
  output projection overlaps the second collective (which pays no extra
  cross-core skew: the first collective already synced the cores); the
  output projection is k-outer so it starts as soon as the first received
  k-tile lands, with bias+store pipelined into the last k-slice.

Compute in bf16 on the PE array (f32 PSUM accumulation, f32 softmax
denominators/normalization). The host pre-transposes x to [dim, b*s] and
pre-casts x/wqkv/wo to bf16 as part of sharding/layout prep.
"""

import sys

sys.path.insert(0, "/opt/trn_rl_repo")

import ml_dtypes
import numpy as np

# Problem constants (hardcoded per harness contract)
B = 2
S = 2048
DIM = 1024
N_HEAD = 16
HD = 64  # head dim
SCALE = HD ** (-0.5)
R = B * S  # 4096 flattened rows
NCORES = 8
HPC = N_HEAD // NCORES  # 2 heads per core
FPC = HPC * HD  # 128 features per core
RPC = R // NCORES  # 512 rows per core (output row slice)

KT = DIM // 128  # 8 k-tiles over the model dim
NKT = S // 128  # 16 key tiles per sequence
NQB = S // 512  # 4 query blocks per sequence
SHIFT = 3  # PV pipeline lag behind QK/exp

_CACHED = {}


def _build_graph():
    import concourse.mybir as mybir
    import concourse.tile as tile
    from concourse import bacc

    nc = bacc.Bacc(
        "TRN2",
        target_bir_lowering=False,
        debug=False,
        num_devices=NCORES,
    )
    return _build_body(nc, mybir, tile)


def _build_body(nc, mybir, tile):
    f32 = mybir.dt.float32
    bf16 = mybir.dt.bfloat16
    EXP = mybir.ActivationFunctionType.Exp

    xt = nc.dram_tensor("xt", [DIM, R], bf16, kind="ExternalInput").ap()
    wqkv = nc.dram_tensor("wqkv", [DIM, 3 * FPC], bf16, kind="ExternalInput").ap()
    bqkv = nc.dram_tensor("bqkv", [3, FPC], f32, kind="ExternalInput").ap()
    wo = nc.dram_tensor("wo", [DIM, DIM], bf16, kind="ExternalInput").ap()
    bo = nc.dram_tensor("bo", [8, 128], f32, kind="ExternalInput").ap()
    out = nc.dram_tensor("out", [DIM, RPC], bf16, kind="ExternalOutput").ap()

    with tile.TileContext(nc) as tc:
        with (
            tc.tile_pool(name="glob", bufs=1) as glob,
            tc.tile_pool(name="dram", bufs=1, space="DRAM") as dram_pool,
        ):
            # ---------------- persistent tiles -------------------------
            ones128 = glob.tile([128, 128], bf16)
            nc.vector.memset(ones128[:], 1.0)
            bias_qkv = glob.tile([128, 2], f32)  # q, k per-partition biases
            vbias = glob.tile([128, 128], f32)  # v bias along free dim
            bias_o = glob.tile([128, 8], f32)
            qT = glob.tile([128, R], bf16)
            kT = glob.tile([128, R], bf16)
            v_nat = glob.tile([128, R], bf16)  # [keys, 2h*64d] per 128-chunk

            warm_in = dram_pool.tile([NCORES, 16], bf16, name="warm_in")
            warm_out = dram_pool.tile([NCORES, 16], bf16, name="warm_out")
            a2a_inA = dram_pool.tile([DIM, RPC // 2], bf16, name="a2a_inA")
            a2a_outA = dram_pool.tile([DIM, RPC // 2], bf16, name="a2a_outA")
            a2a_inB = dram_pool.tile([DIM, RPC // 2], bf16, name="a2a_inB")
            a2a_outB = dram_pool.tile([DIM, RPC // 2], bf16, name="a2a_outB")

            # ---------------- phase 0: weight/bias DMAs, warm a2a ------
            wqkv_sb = []
            for k in range(KT):
                w_t = glob.tile([128, 3 * FPC], bf16, name=f"w_{k}")
                nc.gpsimd.dma_start(out=w_t[:], in_=wqkv[k * 128 : (k + 1) * 128, :])
                wqkv_sb.append(w_t)
            for m in range(2):
                nc.gpsimd.dma_start(
                    out=bias_qkv[:, m : m + 1], in_=bqkv[m : m + 1, :]
                )
            nc.gpsimd.dma_start(
                out=vbias[:], in_=bqkv[2:3, :].to_broadcast((128, 128))
            )
            for m in range(8):
                nc.gpsimd.dma_start(out=bias_o[:, m : m + 1], in_=bo[m : m + 1, :])

            warm_sb = glob.tile([1, 16], bf16)
            nc.vector.memset(warm_sb[:], 1.0)
            # touch Exp immediately so the ~1.3us ACT table load happens
            # during the initial DMA wait, not before the first real exp
            warm_act = glob.tile([1, 16], f32)
            nc.scalar.activation(warm_act[:], warm_sb[:], EXP)
            # dense dummy-matmul burst inside the otherwise-idle input-DMA
            # window: ~3.4us of sustained PE busy flips the HAM clock gate
            # to 8/8 so the projection prefix runs at full clock (one
            # LDWEIGHTS: the stationary never changes)
            with tc.tile_pool(name="jnk", bufs=2, space="PSUM") as jnk_pool:
                for _ in range(90):
                    jp = jnk_pool.tile([128, 128], f32, name="jp", tag="j")
                    nc.tensor.matmul(
                        jp[:], lhsT=ones128[:], rhs=ones128[:],
                        start=True, stop=True,
                    )
            nc.gpsimd.dma_start(out=warm_in[0:1, :], in_=warm_sb[0:1, :])
            nc.gpsimd.dma_start(
                out=warm_in[1:NCORES, :],
                in_=warm_in[0:1, :].to_broadcast((NCORES - 1, 16)),
            )
            nc.gpsimd.collective_compute(
                "AllToAll",
                mybir.AluOpType.bypass,
                replica_groups=[list(range(NCORES))],
                ins=[warm_in[:].opt()],
                outs=[warm_out[:].opt()],
            )
            wo_sb = []
            for k in range(KT):
                w_t = glob.tile([128, DIM], bf16, name=f"wo_{k}")
                nc.gpsimd.dma_start(out=w_t[:], in_=wo[k * 128 : (k + 1) * 128, :])
                wo_sb.append(w_t)

            with tc.tile_pool(name="xTp", bufs=2) as xT_pool:

                def dma_group(g, eng=None, split=False):
                    """DMA one 1024-row group of xt; returns the 8 k-tiles."""
                    eng = eng or nc.sync
                    xg = []
                    for k in range(KT):
                        t = xT_pool.tile(
                            [128, 1024], bf16, name=f"xT_{k}", tag=f"xT{k}"
                        )
                        e = nc.gpsimd if (split and k % 2) else eng
                        e.dma_start(
                            out=t[:],
                            in_=xt[
                                k * 128 : (k + 1) * 128, g * 1024 : (g + 1) * 1024
                            ],
                        )
                        xg.append(t)
                    return xg

                def qk_mms(pp, xg, m, h, ks):
                    for k in ks:
                        nc.tensor.matmul(
                            pp[:],
                            lhsT=wqkv_sb[k][:, m * 128 : (m + 1) * 128],
                            rhs=xg[k][:, h * 512 : (h + 1) * 512],
                            start=(k == 0),
                            stop=(k == KT - 1),
                        )

                def qk_bias(pp, g, m, h):
                    col0 = g * 1024 + h * 512
                    dst = qT if m == 0 else kT
                    nc.vector.tensor_scalar_add(
                        out=dst[:, col0 : col0 + 512],
                        in0=pp[:],
                        scalar1=bias_qkv[:, m : m + 1],
                    )

                def v_mms(vd, xg, c, ks):
                    for k in ks:
                        nc.tensor.matmul(
                            vd[:],
                            lhsT=xg[k][:, c * 128 : (c + 1) * 128],
                            rhs=wqkv_sb[k][:, 256:384],
                            start=(k == 0),
                            stop=(k == KT - 1),
                        )

                def v_bias(vd, g, c):
                    chunk = g * 8 + c
                    nc.vector.tensor_add(
                        out=v_nat[:, chunk * 128 : (chunk + 1) * 128],
                        in0=vd[:],
                        in1=vbias[:],
                    )

                # -------- phase 1 prefix: just enough to start block 0 --
                xgs = {}
                with (
                    tc.tile_pool(name="pp1", bufs=2, space="PSUM") as pp1_pool,
                    tc.tile_pool(name="vd1", bufs=1, space="PSUM") as vd1_pool,
                ):
                    xgs[0] = dma_group(0)
                    xgs[1] = dma_group(1)
                    # mini k-round: key tile 0 only, so the first QK/exp can
                    # issue several microseconds before the full rounds land
                    ppm = pp1_pool.tile([128, 128], f32, name="ppm", tag="pp")
                    for k in range(KT):
                        nc.tensor.matmul(
                            ppm[:],
                            lhsT=wqkv_sb[k][:, 128:256],
                            rhs=xgs[0][k][:, 0:128],
                            start=(k == 0),
                            stop=(k == KT - 1),
                        )
                    nc.vector.tensor_scalar_add(
                        out=kT[:, 0:128], in0=ppm[:], scalar1=bias_qkv[:, 1:2]
                    )
                    pp = pp1_pool.tile([128, 512], f32, name="pp", tag="pp")
                    qk_mms(pp, xgs[0], 0, 0, range(KT))  # q, rows 0-511
                    qk_bias(pp, 0, 0, 0)
                    pp = pp1_pool.tile([128, 512], f32, name="pp", tag="pp")
                    qk_mms(pp, xgs[0], 1, 0, range(KT))  # k, rows 0-511
                    qk_bias(pp, 0, 1, 0)
                    vd = vd1_pool.tile([128, 128], f32, name="vd", tag="vd")
                    v_mms(vd, xgs[0], 0, range(KT))  # v chunk 0
                    v_bias(vd, 0, 0)

                # -------- phase 2: attention + interleaved projection --
                with (
                    tc.tile_pool(name="pstp", bufs=2, space="PSUM") as pst_pool,
                    tc.tile_pool(name="pop", bufs=1, space="PSUM") as po_pool,
                    tc.tile_pool(name="denp", bufs=1, space="PSUM") as den_pool,
                    tc.tile_pool(name="pp2", bufs=1, space="PSUM") as pp2_pool,
                    tc.tile_pool(name="ptp", bufs=12) as pt_pool,
                    tc.tile_pool(name="pairp", bufs=2) as pair_pool,
                    tc.tile_pool(name="quadp", bufs=2) as quad_pool,
                    tc.tile_pool(name="hexp", bufs=2) as hex_pool,
                    tc.tile_pool(name="recipp", bufs=2) as recip_pool,
                    tc.tile_pool(name="oTsp", bufs=2) as oTs_pool,
                ):
                    st = {"pp": None, "vd": None, "pending": None}

                    # ---- deferred projection units (deadline-ordered) --
                    def u_dma(g):
                        return lambda: xgs.__setitem__(g, dma_group(g, nc.gpsimd))

                    def u_round_start(g, m, h, pool):
                        def f():
                            st["pp"] = pool.tile(
                                [128, 512], f32, name="pp", tag="pp"
                            )
                            qk_mms(st["pp"], xgs[g], m, h, range(2))

                        return f

                    def u_round_mid(g, m, h, ks):
                        return lambda: qk_mms(st["pp"], xgs[g], m, h, ks)

                    def u_round_end(g, m, h):
                        def f():
                            qk_mms(st["pp"], xgs[g], m, h, range(6, 8))
                            qk_bias(st["pp"], g, m, h)

                        return f

                    def round_units(g, m, h, pool):
                        return [
                            u_round_start(g, m, h, pool),
                            u_round_mid(g, m, h, range(2, 4)),
                            u_round_mid(g, m, h, range(4, 6)),
                            u_round_end(g, m, h),
                        ]

                    def u_v_a(g, c, pool):
                        def f():
                            st["vd"] = pool.tile(
                                [128, 128], f32, name="vd", tag="pp"
                            )
                            v_mms(st["vd"], xgs[g], c, range(4))

                        return f

                    def u_v_b(g, c):
                        def f():
                            v_mms(st["vd"], xgs[g], c, range(4, 8))
                            v_bias(st["vd"], g, c)

                        return f

                    def v_units(g, c, pool):
                        return [u_v_a(g, c, pool), u_v_b(g, c)]

                    p2 = pp2_pool
                    units = []
                    # batch-0 remainder, deadline-interleaved for block 0
                    # (3 pops/kt): v chunk c is needed by PV at kt c+SHIFT;
                    # k-round (g,h) covers key tiles g*8+h*4 .. +3.
                    units += v_units(0, 1, p2) + v_units(0, 2, p2)
                    units += round_units(0, 1, 1, p2)  # k rows 512-1023
                    units += v_units(0, 3, p2) + v_units(0, 4, p2)
                    units += round_units(1, 1, 0, p2)  # k rows 1024-1535
                    units += v_units(0, 5, p2) + v_units(0, 6, p2)
                    units += round_units(1, 1, 1, p2)  # k rows 1536-2047
                    units += v_units(0, 7, p2)
                    for c in range(8):
                        units += v_units(1, c, p2)  # v rows 1024-2047
                    units += round_units(0, 0, 1, p2)  # q for block 1
                    units += round_units(1, 0, 0, p2)  # q for block 2
                    units += round_units(1, 0, 1, p2)  # q for block 3
                    # batch 1: k and v (needed by block 4), q(g2,h0) too
                    units += [u_dma(2)]
                    units += round_units(2, 1, 0, p2) + round_units(2, 1, 1, p2)
                    for c in range(4):
                        units += v_units(2, c, p2)
                    units += [u_dma(3)]
                    for c in range(4, 8):
                        units += v_units(2, c, p2)
                    units += round_units(3, 1, 0, p2) + round_units(3, 1, 1, p2)
                    for c in range(8):
                        units += v_units(3, c, p2)
                    units += round_units(2, 0, 0, p2)  # q for block 4
                    # popped during blocks 4-6:
                    late_units = (
                        round_units(2, 0, 1, p2)  # q block 5
                        + round_units(3, 0, 0, p2)  # q block 6
                        + round_units(3, 0, 1, p2)  # q block 7
                    )
                    units.reverse()
                    late_units.reverse()

                    def emit_pv(blk, kt, pts, po, tree):
                        b = blk // NQB
                        off = (b * NKT + kt) * 128
                        pt = pts[kt]
                        nc.tensor.matmul(
                            po[0:64, :],
                            lhsT=v_nat[:, off : off + 64],
                            rhs=pt[:, 0:512],
                            start=(kt == 0),
                            stop=(kt == NKT - 1),
                            tile_position=(0, 0),
                        )
                        nc.tensor.matmul(
                            po[64:128, :],
                            lhsT=v_nat[:, off + 64 : off + 128],
                            rhs=pt[:, 512:1024],
                            start=(kt == 0),
                            stop=(kt == NKT - 1),
                            tile_position=(0, 64),
                        )
                        # bf16 reduction tree toward the denominators
                        if kt % 2 == 1:
                            pr = pair_pool.tile(
                                [128, 1024], bf16, name="pair", tag="pair"
                            )
                            nc.vector.tensor_add(
                                out=pr[:], in0=pts[kt - 1][:], in1=pt[:]
                            )
                            tree["pair"].append(pr)
                        if kt % 4 == 3:
                            qd = quad_pool.tile(
                                [128, 1024], bf16, name="quad", tag="quad"
                            )
                            nc.vector.tensor_add(
                                out=qd[:],
                                in0=tree["pair"][-2][:],
                                in1=tree["pair"][-1][:],
                            )
                            tree["quad"].append(qd)
                        if kt % 8 == 7:
                            hx = hex_pool.tile(
                                [128, 1024], bf16, name="hex", tag="hex"
                            )
                            nc.vector.tensor_add(
                                out=hx[:],
                                in0=tree["quad"][-2][:],
                                in1=tree["quad"][-1][:],
                            )
                            tree["hex"].append(hx)

                    def tail_a1(blk, pts, po, tree):
                        emit_pv(blk, NKT - 3, pts, po, tree)

                    def tail_a2(blk, pts, po, tree):
                        emit_pv(blk, NKT - 2, pts, po, tree)
                        emit_pv(blk, NKT - 1, pts, po, tree)

                    def tail_b(blk, pts, po, tree):
                        # denominator part 1: hex0 into both halves (start)
                        dn = den_pool.tile([128, 1024], f32, name="den", tag="den")
                        st["den"] = dn
                        hx0 = tree["hex"][0]
                        for half in range(2):
                            c0 = half * 512
                            nc.tensor.matmul(
                                dn[:, c0 : c0 + 512],
                                lhsT=ones128[:],
                                rhs=hx0[:, c0 : c0 + 512],
                                start=True,
                                stop=False,
                            )

                    def tail_c(blk, pts, po, tree):
                        dn = st["den"]
                        hx1 = tree["hex"][1]
                        for half in range(2):
                            c0 = half * 512
                            nc.tensor.matmul(
                                dn[:, c0 : c0 + 512],
                                lhsT=ones128[:],
                                rhs=hx1[:, c0 : c0 + 512],
                                start=False,
                                stop=True,
                            )
                        recip = recip_pool.tile(
                            [128, 1024], f32, name="recip", tag="rc"
                        )
                        nc.vector.reciprocal_approx_fast(out=recip[:], in_=dn[:])
                        oTs = oTs_pool.tile([128, 512], bf16, name="oTs", tag="oTs")
                        nc.vector.tensor_mul(
                            out=oTs[0:64, :],
                            in0=po[0:64, :],
                            in1=recip[0:64, 0:512],
                        )
                        nc.vector.tensor_mul(
                            out=oTs[64:128, :],
                            in0=po[64:128, :],
                            in1=recip[64:128, 512:1024],
                        )
                        nc.sync.dma_start(
                            out=a2a_inA[blk * 128 : (blk + 1) * 128, :],
                            in_=oTs[:, 0 : RPC // 2],
                        )
                        nc.sync.dma_start(
                            out=a2a_inB[blk * 128 : (blk + 1) * 128, :],
                            in_=oTs[:, RPC // 2 : RPC],
                        )

                    for b in range(B):
                        for qb in range(NQB):
                            blk = b * NQB + qb
                            q0 = b * S + qb * 512
                            pts = []
                            tree = {"pair": [], "quad": [], "hex": []}
                            po = None
                            for kt in range(NKT):
                                k0 = b * S + kt * 128
                                pst = pst_pool.tile(
                                    [128, 1024], f32, name="pst", tag="st"
                                )
                                for hh in range(HPC):
                                    nc.tensor.matmul(
                                        pst[:, hh * 512 : (hh + 1) * 512],
                                        lhsT=kT[
                                            hh * 64 : (hh + 1) * 64, k0 : k0 + 128
                                        ],
                                        rhs=qT[
                                            hh * 64 : (hh + 1) * 64, q0 : q0 + 512
                                        ],
                                        start=True,
                                        stop=True,
                                        tile_position=(hh * 64, 0),
                                    )
                                pt = pt_pool.tile(
                                    [128, 1024], bf16, name="ptile", tag="pt"
                                )
                                nc.scalar.activation(
                                    pt[:], pst[:], EXP, scale=SCALE
                                )
                                pts.append(pt)
                                pend = st["pending"]
                                if kt == 0 and pend:
                                    tail_a1(*pend)
                                elif kt == 1 and pend:
                                    tail_a2(*pend)
                                elif kt == 2 and pend:
                                    tail_b(*pend)
                                elif kt == 3 and pend:
                                    tail_c(*pend)
                                    st["pending"] = None
                                if blk == 7 and kt == 13:
                                    # last block: its hex0 is ready (kt10) so
                                    # the first denominator pair can run now,
                                    # shortening the pre-exchange tail chain
                                    dn = den_pool.tile(
                                        [128, 1024], f32, name="den", tag="den"
                                    )
                                    st["den"] = dn
                                    hx0 = tree["hex"][0]
                                    for half in range(2):
                                        c0 = half * 512
                                        nc.tensor.matmul(
                                            dn[:, c0 : c0 + 512],
                                            lhsT=ones128[:],
                                            rhs=hx0[:, c0 : c0 + 512],
                                            start=True,
                                            stop=False,
                                        )
                                if kt == SHIFT:
                                    po = po_pool.tile(
                                        [128, 512], f32, name="po", tag="po"
                                    )
                                if kt >= SHIFT:
                                    emit_pv(blk, kt - SHIFT, pts, po, tree)
                                # deadline-paced unit pops, kept away from the
                                # block-boundary key-tiles that carry the
                                # previous block's denominator/normalize work
                                npop = 0
                                if blk == 0:
                                    npop = 3 if kt < 14 else 2
                                elif blk < 4:
                                    npop = 0 if kt < 4 else (2 if kt < 14 else 1)
                                elif blk < 7:
                                    npop = 1 if kt in (5, 7, 9, 11) else 0
                                for _ in range(npop):
                                    if blk < 4 and units:
                                        units.pop()()
                                    elif late_units:
                                        late_units.pop()()
                            st["pending"] = (blk, pts, po, tree)
                    # flush the last block (denA already ran at kt13)
                    tail_a1(*st["pending"])
                    tail_a2(*st["pending"])
                    tail_c(*st["pending"])
                    st["pending"] = None
                    while units:
                        units.pop()()
                    while late_units:
                        late_units.pop()()
                    for buf_in, buf_out in ((a2a_inA, a2a_outA), (a2a_inB, a2a_outB)):
                        nc.gpsimd.collective_compute(
                            "AllToAll",
                            mybir.AluOpType.bypass,
                            replica_groups=[list(range(NCORES))],
                            ins=[buf_in[:].opt()],
                            outs=[buf_out[:].opt()],
                        )

            # ---------------- phase 3: output projection ---------------
            with (
                tc.tile_pool(name="ots", bufs=1) as ots_pool,
                tc.tile_pool(name="psout", bufs=1, space="PSUM") as ps_out,
                tc.tile_pool(name="outt", bufs=2) as out_pool,
            ):
                pouts = [
                    ps_out.tile([128, 256], f32, name=f"pout{m}", tag=f"po{m}")
                    for m in range(8)
                ]
                for half, buf_out in ((0, a2a_outA), (1, a2a_outB)):
                    for k in range(KT):
                        o_t = ots_pool.tile(
                            [128, RPC // 2], bf16, name=f"oTs_{k}", tag=f"ot{k}"
                        )
                        nc.sync.dma_start(
                            out=o_t[:], in_=buf_out[k * 128 : (k + 1) * 128, :]
                        )
                        last = k == KT - 1
                        for m in range(8):
                            nc.tensor.matmul(
                                pouts[m][:],
                                lhsT=wo_sb[k][:, m * 128 : (m + 1) * 128],
                                rhs=o_t[:],
                                start=(k == 0),
                                stop=last,
                            )
                            if last:
                                o_sb = out_pool.tile(
                                    [128, 256], bf16, name="o_sb", tag="o_sb"
                                )
                                nc.vector.tensor_scalar_add(
                                    out=o_sb[:],
                                    in0=pouts[m][:],
                                    scalar1=bias_o[:, m : m + 1],
                                )
                                nc.sync.dma_start(
                                    out=out[
                                        m * 128 : (m + 1) * 128,
                                        half * 256 : (half + 1) * 256,
                                    ],
                                    in_=o_sb[:],
                                )

    nc.compile()
    return nc


def _get_graph():
    if "nc" not in _CACHED:
        _CACHED["nc"] = _build_graph()
    return _CACHED["nc"]


def _make_in_maps(x, wqkv, bqkv, wo, bo):
    bf = ml_dtypes.bfloat16
    x2 = np.asarray(x, dtype=np.float32).reshape(R, DIM)
    xt = np.ascontiguousarray(x2.T.astype(bf))  # [dim, b*s] bf16
    wqkv = np.asarray(wqkv, dtype=np.float32)
    bqkv = np.asarray(bqkv, dtype=np.float32)
    wo16 = np.ascontiguousarray(np.asarray(wo, dtype=np.float32).astype(bf))
    bo_f = np.ascontiguousarray(np.asarray(bo, dtype=np.float32).reshape(8, 128))

    in_maps = []
    for c in range(NCORES):
        w_s = np.ascontiguousarray(
            np.concatenate(
                [
                    wqkv[:, c * FPC : (c + 1) * FPC],
                    wqkv[:, DIM + c * FPC : DIM + (c + 1) * FPC],
                    wqkv[:, 2 * DIM + c * FPC : 2 * DIM + (c + 1) * FPC],
                ],
                axis=1,
            ).astype(bf)
        )
        b_s = np.ascontiguousarray(
            np.stack(
                [
                    bqkv[c * FPC : (c + 1) * FPC],
                    bqkv[DIM + c * FPC : DIM + (c + 1) * FPC],
                    bqkv[2 * DIM + c * FPC : 2 * DIM + (c + 1) * FPC],
                ],
                axis=0,
            )
        )
        in_maps.append({"xt": xt, "wqkv": w_s, "bqkv": b_s, "wo": wo16, "bo": bo_f})
    return in_maps


def kernel(x, wqkv, bqkv, wo, bo):
    from concourse.bass_utils import run_bass_kernel_spmd

    nc = _get_graph()
    in_maps = _make_in_maps(x, wqkv, bqkv, wo, bo)
    res = run_bass_kernel_spmd(nc, in_maps, core_ids=list(range(NCORES)))
    outs = [res.results[c]["out"] for c in range(NCORES)]  # each [1024, 512]
    full = np.concatenate([o.T for o in outs], axis=0)  # [4096, 1024]
    return np.ascontiguousarray(full.reshape(B, S, DIM)).astype(np.float32)


# revision 36
# speedup vs baseline: 1.1531x; 1.0707x over previous
"""Distributed multi-head attention kernel for 8 TRN2 NeuronCores.

Sharding: tensor-parallel over heads (2 heads/core). Per core: qkv projection
for its 128 features, attention for its 2 heads, AllToAll exchange, then
row-parallel output projection (each core produces a transposed 512-row slice
of the final output); host reassembles.

Structure:
- V is projected directly in [rows, feat] layout (stationary = x k-tile,
  M = 128 rows): no PE transposes anywhere.
- PV uses column-split tile_position packing: both heads' PV matmuls run
  concurrently in column halves of the PE array (one 512-cycle pass per key
  tile instead of two M=65 passes).
- Softmax denominators come from a bf16 pair/quad/hex DVE reduction tree over
  the exp tiles plus 4 all-ones matmuls per block; reciprocals via the DVE
  reciprocal_approx_fast custom op (the scalar engine runs nothing but the
  128 exp tiles); normalization is two plain DVE multiplies whose recip
  operand layout matches po's partition split.
- The attention phase is ACT(exp)-bound, so everything else is threaded
  through its PE slack: the emission is software-pipelined (QK/exp of tile
  kt runs SHIFT tiles ahead of PV), the previous block's denominator/
  normalize work is spread over the next block's first tiles, and the whole
  remaining projection work (q for later blocks, all of batch 1) is emitted
  as fine-grained units popped between key tiles with deadline ordering.
- The exchange is split into two half-query AllToAlls so the first half's
  output projection overlaps the second collective (which pays no extra
  cross-core skew: the first collective already synced the cores); the
  output projection is k-outer so it starts as soon as the first received
  k-tile lands, with bias+store pipelined into the last k-slice.

Compute in bf16 on the PE array (f32 PSUM accumulation, f32 softmax
denominators/normalization). The host pre-transposes x to [dim, b*s] and
pre-casts x/wqkv/wo to bf16 as part of sharding/layout prep.
"""

import sys

sys.path.insert(0, "/opt/trn_rl_repo")

import ml_dtypes
import numpy as np

# Problem constants (hardcoded per harness contract)
B = 2
S = 2048
DIM = 1024
N_HEAD = 16
HD = 64  # head dim
SCALE = HD ** (-0.5)
R = B * S  # 4096 flattened rows
NCORES = 8
HPC = N_HEAD // NCORES  # 2 heads per core
FPC = HPC * HD  # 128 features per core
RPC = R // NCORES  # 512 rows per core (output row slice)

KT = DIM // 128  # 8 k-tiles over the model dim
NKT = S // 128  # 16 key tiles per sequence
NQB = S // 512  # 4 query blocks per sequence
SHIFT = 3  # PV pipeline lag behind QK/exp

_CACHED = {}


def _build_graph():
    import concourse.mybir as mybir
    import concourse.tile as tile
    from concourse import bacc

    nc = bacc.Bacc(
        "TRN2",
        target_bir_lowering=False,
        debug=False,
        num_devices=NCORES,
    )
    return _build_body(nc, mybir, tile)


def _build_body(nc, mybir, tile):
    f32 = mybir.dt.float32
    bf16 = mybir.dt.bfloat16
    EXP = mybir.ActivationFunctionType.Exp

    xt = nc.dram_tensor("xt", [DIM, R], bf16, kind="ExternalInput").ap()
    wqkv = nc.dram_tensor("wqkv", [DIM, 3 * FPC], bf16, kind="ExternalInput").ap()
    bqkv = nc.dram_tensor("bqkv", [3, FPC], f32, kind="ExternalInput").ap()
    wo = nc.dram_tensor("wo", [DIM, DIM], bf16, kind="ExternalInput").ap()
    bo = nc.dram_tensor("bo", [8, 128], f32, kind="ExternalInput").ap()
    out = nc.dram_tensor("out", [DIM, RPC], bf16, kind="ExternalOutput").ap()

    with tile.TileContext(nc) as tc:
        with (
            tc.tile_pool(name="glob", bufs=1) as glob,
            tc.tile_pool(name="dram", bufs=1, space="DRAM") as dram_pool,
        ):
            # ---------------- persistent tiles -------------------------
            ones128 = glob.tile([128, 128], bf16)
            nc.vector.memset(ones128[:], 1.0)
            bias_qkv = glob.tile([128, 2], f32)  # q, k per-partition biases
            vbias = glob.tile([128, 128], f32)  # v bias along free dim
            bias_o = glob.tile([128, 8], f32)
            qT = glob.tile([128, R], bf16)
            kT = glob.tile([128, R], bf16)
            v_nat = glob.tile([128, R], bf16)  # [keys, 2h*64d] per 128-chunk

            warm_in = dram_pool.tile([NCORES, 16], bf16, name="warm_in")
            warm_out = dram_pool.tile([NCORES, 16], bf16, name="warm_out")
            a2a_inA = dram_pool.tile([DIM, RPC // 2], bf16, name="a2a_inA")
            a2a_outA = dram_pool.tile([DIM, RPC // 2], bf16, name="a2a_outA")
            a2a_inB = dram_pool.tile([DIM, RPC // 2], bf16, name="a2a_inB")
            a2a_outB = dram_pool.tile([DIM, RPC // 2], bf16, name="a2a_outB")

            # ---------------- phase 0: weight/bias DMAs, warm a2a ------
            wqkv_sb = []
            for k in range(KT):
                w_t = glob.tile([128, 3 * FPC], bf16, name=f"w_{k}")
                nc.gpsimd.dma_start(out=w_t[:], in_=wqkv[k * 128 : (k + 1) * 128, :])
                wqkv_sb.append(w_t)
            for m in range(2):
                nc.gpsimd.dma_start(
                    out=bias_qkv[:, m : m + 1], in_=bqkv[m : m + 1, :]
                )
            nc.gpsimd.dma_start(
                out=vbias[:], in_=bqkv[2:3, :].to_broadcast((128, 128))
            )
            for m in range(8):
                nc.gpsimd.dma_start(out=bias_o[:, m : m + 1], in_=bo[m : m + 1, :])

            warm_sb = glob.tile([1, 16], bf16)
            nc.vector.memset(warm_sb[:], 1.0)
            # touch Exp immediately so the ~1.3us ACT table load happens
            # during the initial DMA wait, not before the first real exp
            warm_act = glob.tile([1, 16], f32)
            nc.scalar.activation(warm_act[:], warm_sb[:], EXP)
            nc.gpsimd.dma_start(out=warm_in[0:1, :], in_=warm_sb[0:1, :])
            nc.gpsimd.dma_start(
                out=warm_in[1:NCORES, :],
                in_=warm_in[0:1, :].to_broadcast((NCORES - 1, 16)),
            )
            nc.gpsimd.collective_compute(
                "AllToAll",
                mybir.AluOpType.bypass,
                replica_groups=[list(range(NCORES))],
                ins=[warm_in[:].opt()],
                outs=[warm_out[:].opt()],
            )
            wo_sb = []
            for k in range(KT):
                w_t = glob.tile([128, DIM], bf16, name=f"wo_{k}")
                nc.gpsimd.dma_start(out=w_t[:], in_=wo[k * 128 : (k + 1) * 128, :])
                wo_sb.append(w_t)

            with tc.tile_pool(name="xTp", bufs=2) as xT_pool:

                def dma_group(g, eng=None, split=False):
                    """DMA one 1024-row group of xt; returns the 8 k-tiles."""
                    eng = eng or nc.sync
                    xg = []
                    for k in range(KT):
                        t = xT_pool.tile(
                            [128, 1024], bf16, name=f"xT_{k}", tag=f"xT{k}"
                        )
                        e = nc.gpsimd if (split and k % 2) else eng
                        e.dma_start(
                            out=t[:],
                            in_=xt[
                                k * 128 : (k + 1) * 128, g * 1024 : (g + 1) * 1024
                            ],
                        )
                        xg.append(t)
                    return xg

                def qk_mms(pp, xg, m, h, ks):
                    for k in ks:
                        nc.tensor.matmul(
                            pp[:],
                            lhsT=wqkv_sb[k][:, m * 128 : (m + 1) * 128],
                            rhs=xg[k][:, h * 512 : (h + 1) * 512],
                            start=(k == 0),
                            stop=(k == KT - 1),
                        )

                def qk_bias(pp, g, m, h):
                    col0 = g * 1024 + h * 512
                    dst = qT if m == 0 else kT
                    nc.vector.tensor_scalar_add(
                        out=dst[:, col0 : col0 + 512],
                        in0=pp[:],
                        scalar1=bias_qkv[:, m : m + 1],
                    )

                def v_mms(vd, xg, c, ks):
                    for k in ks:
                        nc.tensor.matmul(
                            vd[:],
                            lhsT=xg[k][:, c * 128 : (c + 1) * 128],
                            rhs=wqkv_sb[k][:, 256:384],
                            start=(k == 0),
                            stop=(k == KT - 1),
                        )

                def v_bias(vd, g, c):
                    chunk = g * 8 + c
                    nc.vector.tensor_add(
                        out=v_nat[:, chunk * 128 : (chunk + 1) * 128],
                        in0=vd[:],
                        in1=vbias[:],
                    )

                # -------- phase 1 prefix: just enough to start block 0 --
                xgs = {}
                with (
                    tc.tile_pool(name="pp1", bufs=2, space="PSUM") as pp1_pool,
                    tc.tile_pool(name="vd1", bufs=1, space="PSUM") as vd1_pool,
                ):
                    xgs[0] = dma_group(0)
                    xgs[1] = dma_group(1)
                    # mini k-round: key tile 0 only, so the first QK/exp can
                    # issue several microseconds before the full rounds land
                    ppm = pp1_pool.tile([128, 128], f32, name="ppm", tag="pp")
                    for k in range(KT):
                        nc.tensor.matmul(
                            ppm[:],
                            lhsT=wqkv_sb[k][:, 128:256],
                            rhs=xgs[0][k][:, 0:128],
                            start=(k == 0),
                            stop=(k == KT - 1),
                        )
                    nc.vector.tensor_scalar_add(
                        out=kT[:, 0:128], in0=ppm[:], scalar1=bias_qkv[:, 1:2]
                    )
                    pp = pp1_pool.tile([128, 512], f32, name="pp", tag="pp")
                    qk_mms(pp, xgs[0], 0, 0, range(KT))  # q, rows 0-511
                    qk_bias(pp, 0, 0, 0)
                    pp = pp1_pool.tile([128, 512], f32, name="pp", tag="pp")
                    qk_mms(pp, xgs[0], 1, 0, range(KT))  # k, rows 0-511
                    qk_bias(pp, 0, 1, 0)
                    vd = vd1_pool.tile([128, 128], f32, name="vd", tag="vd")
                    v_mms(vd, xgs[0], 0, range(KT))  # v chunk 0
                    v_bias(vd, 0, 0)

                # -------- phase 2: attention + interleaved projection --
                with (
                    tc.tile_pool(name="pstp", bufs=2, space="PSUM") as pst_pool,
                    tc.tile_pool(name="pop", bufs=1, space="PSUM") as po_pool,
                    tc.tile_pool(name="denp", bufs=1, space="PSUM") as den_pool,
                    tc.tile_pool(name="pp2", bufs=1, space="PSUM") as pp2_pool,
                    tc.tile_pool(name="ptp", bufs=12) as pt_pool,
                    tc.tile_pool(name="pairp", bufs=2) as pair_pool,
                    tc.tile_pool(name="quadp", bufs=2) as quad_pool,
                    tc.tile_pool(name="hexp", bufs=2) as hex_pool,
                    tc.tile_pool(name="recipp", bufs=2) as recip_pool,
                    tc.tile_pool(name="oTsp", bufs=2) as oTs_pool,
                ):
                    st = {"pp": None, "vd": None, "pending": None}

                    # ---- deferred projection units (deadline-ordered) --
                    def u_dma(g):
                        return lambda: xgs.__setitem__(g, dma_group(g, nc.gpsimd))

                    def u_round_start(g, m, h, pool):
                        def f():
                            st["pp"] = pool.tile(
                                [128, 512], f32, name="pp", tag="pp"
                            )
                            qk_mms(st["pp"], xgs[g], m, h, range(2))

                        return f

                    def u_round_mid(g, m, h, ks):
                        return lambda: qk_mms(st["pp"], xgs[g], m, h, ks)

                    def u_round_end(g, m, h):
                        def f():
                            qk_mms(st["pp"], xgs[g], m, h, range(6, 8))
                            qk_bias(st["pp"], g, m, h)

                        return f

                    def round_units(g, m, h, pool):
                        return [
                            u_round_start(g, m, h, pool),
                            u_round_mid(g, m, h, range(2, 4)),
                            u_round_mid(g, m, h, range(4, 6)),
                            u_round_end(g, m, h),
                        ]

                    def u_v_a(g, c, pool):
                        def f():
                            st["vd"] = pool.tile(
                                [128, 128], f32, name="vd", tag="pp"
                            )
                            v_mms(st["vd"], xgs[g], c, range(4))

                        return f

                    def u_v_b(g, c):
                        def f():
                            v_mms(st["vd"], xgs[g], c, range(4, 8))
                            v_bias(st["vd"], g, c)

                        return f

                    def v_units(g, c, pool):
                        return [u_v_a(g, c, pool), u_v_b(g, c)]

                    p2 = pp2_pool
                    units = []
                    # batch-0 remainder, deadline-interleaved for block 0
                    # (3 pops/kt): v chunk c is needed by PV at kt c+SHIFT;
                    # k-round (g,h) covers key tiles g*8+h*4 .. +3.
                    units += v_units(0, 1, p2) + v_units(0, 2, p2)
                    units += round_units(0, 1, 1, p2)  # k rows 512-1023
                    units += v_units(0, 3, p2) + v_units(0, 4, p2)
                    units += round_units(1, 1, 0, p2)  # k rows 1024-1535
                    units += v_units(0, 5, p2) + v_units(0, 6, p2)
                    units += round_units(1, 1, 1, p2)  # k rows 1536-2047
                    units += v_units(0, 7, p2)
                    for c in range(8):
                        units += v_units(1, c, p2)  # v rows 1024-2047
                    units += round_units(0, 0, 1, p2)  # q for block 1
                    units += round_units(1, 0, 0, p2)  # q for block 2
                    units += round_units(1, 0, 1, p2)  # q for block 3
                    # batch 1: k and v (needed by block 4), q(g2,h0) too
                    units += [u_dma(2)]
                    units += round_units(2, 1, 0, p2) + round_units(2, 1, 1, p2)
                    for c in range(4):
                        units += v_units(2, c, p2)
                    units += [u_dma(3)]
                    for c in range(4, 8):
                        units += v_units(2, c, p2)
                    units += round_units(3, 1, 0, p2) + round_units(3, 1, 1, p2)
                    for c in range(8):
                        units += v_units(3, c, p2)
                    units += round_units(2, 0, 0, p2)  # q for block 4
                    # popped during blocks 4-6:
                    late_units = (
                        round_units(2, 0, 1, p2)  # q block 5
                        + round_units(3, 0, 0, p2)  # q block 6
                        + round_units(3, 0, 1, p2)  # q block 7
                    )
                    units.reverse()
                    late_units.reverse()

                    def emit_pv(blk, kt, pts, po, tree):
                        b = blk // NQB
                        off = (b * NKT + kt) * 128
                        pt = pts[kt]
                        nc.tensor.matmul(
                            po[0:64, :],
                            lhsT=v_nat[:, off : off + 64],
                            rhs=pt[:, 0:512],
                            start=(kt == 0),
                            stop=(kt == NKT - 1),
                            tile_position=(0, 0),
                        )
                        nc.tensor.matmul(
                            po[64:128, :],
                            lhsT=v_nat[:, off + 64 : off + 128],
                            rhs=pt[:, 512:1024],
                            start=(kt == 0),
                            stop=(kt == NKT - 1),
                            tile_position=(0, 64),
                        )
                        # bf16 reduction tree toward the denominators
                        if kt % 2 == 1:
                            pr = pair_pool.tile(
                                [128, 1024], bf16, name="pair", tag="pair"
                            )
                            nc.vector.tensor_add(
                                out=pr[:], in0=pts[kt - 1][:], in1=pt[:]
                            )
                            tree["pair"].append(pr)
                        if kt % 4 == 3:
                            qd = quad_pool.tile(
                                [128, 1024], bf16, name="quad", tag="quad"
                            )
                            nc.vector.tensor_add(
                                out=qd[:],
                                in0=tree["pair"][-2][:],
                                in1=tree["pair"][-1][:],
                            )
                            tree["quad"].append(qd)
                        if kt % 8 == 7:
                            hx = hex_pool.tile(
                                [128, 1024], bf16, name="hex", tag="hex"
                            )
                            nc.vector.tensor_add(
                                out=hx[:],
                                in0=tree["quad"][-2][:],
                                in1=tree["quad"][-1][:],
                            )
                            tree["hex"].append(hx)

                    def tail_a1(blk, pts, po, tree):
                        emit_pv(blk, NKT - 3, pts, po, tree)

                    def tail_a2(blk, pts, po, tree):
                        emit_pv(blk, NKT - 2, pts, po, tree)
                        emit_pv(blk, NKT - 1, pts, po, tree)

                    def tail_b(blk, pts, po, tree):
                        # denominator part 1: hex0 into both halves (start)
                        dn = den_pool.tile([128, 1024], f32, name="den", tag="den")
                        st["den"] = dn
                        hx0 = tree["hex"][0]
                        for half in range(2):
                            c0 = half * 512
                            nc.tensor.matmul(
                                dn[:, c0 : c0 + 512],
                                lhsT=ones128[:],
                                rhs=hx0[:, c0 : c0 + 512],
                                start=True,
                                stop=False,
                            )

                    def tail_c(blk, pts, po, tree):
                        dn = st["den"]
                        hx1 = tree["hex"][1]
                        for half in range(2):
                            c0 = half * 512
                            nc.tensor.matmul(
                                dn[:, c0 : c0 + 512],
                                lhsT=ones128[:],
                                rhs=hx1[:, c0 : c0 + 512],
                                start=False,
                                stop=True,
                            )
                        recip = recip_pool.tile(
                            [128, 1024], f32, name="recip", tag="rc"
                        )
                        nc.vector.reciprocal_approx_fast(out=recip[:], in_=dn[:])
                        oTs = oTs_pool.tile([128, 512], bf16, name="oTs", tag="oTs")
                        nc.vector.tensor_mul(
                            out=oTs[0:64, :],
                            in0=po[0:64, :],
                            in1=recip[0:64, 0:512],
                        )
                        nc.vector.tensor_mul(
                            out=oTs[64:128, :],
                            in0=po[64:128, :],
                            in1=recip[64:128, 512:1024],
                        )
                        nc.sync.dma_start(
                            out=a2a_inA[blk * 128 : (blk + 1) * 128, :],
                            in_=oTs[:, 0 : RPC // 2],
                        )
                        nc.sync.dma_start(
                            out=a2a_inB[blk * 128 : (blk + 1) * 128, :],
                            in_=oTs[:, RPC // 2 : RPC],
                        )

                    for b in range(B):
                        for qb in range(NQB):
                            blk = b * NQB + qb
                            q0 = b * S + qb * 512
                            pts = []
                            tree = {"pair": [], "quad": [], "hex": []}
                            po = None
                            for kt in range(NKT):
                                k0 = b * S + kt * 128
                                pst = pst_pool.tile(
                                    [128, 1024], f32, name="pst", tag="st"
                                )
                                for hh in range(HPC):
                                    nc.tensor.matmul(
                                        pst[:, hh * 512 : (hh + 1) * 512],
                                        lhsT=kT[
                                            hh * 64 : (hh + 1) * 64, k0 : k0 + 128
                                        ],
                                        rhs=qT[
                                            hh * 64 : (hh + 1) * 64, q0 : q0 + 512
                                        ],
                                        start=True,
                                        stop=True,
                                        tile_position=(hh * 64, 0),
                                    )
                                pt = pt_pool.tile(
                                    [128, 1024], bf16, name="ptile", tag="pt"
                                )
                                nc.scalar.activation(
                                    pt[:], pst[:], EXP, scale=SCALE
                                )
                                pts.append(pt)
                                pend = st["pending"]
                                if kt == 0 and pend:
                                    tail_a1(*pend)
                                elif kt == 1 and pend:
                                    tail_a2(*pend)
                                elif kt == 2 and pend:
                                    tail_b(*pend)
                                elif kt == 3 and pend:
                                    tail_c(*pend)
                                    st["pending"] = None
                                if blk == 7 and kt == 13:
                                    # last block: its hex0 is ready (kt10) so
                                    # the first denominator pair can run now,
                                    # shortening the pre-exchange tail chain
                                    dn = den_pool.tile(
                                        [128, 1024], f32, name="den", tag="den"
                                    )
                                    st["den"] = dn
                                    hx0 = tree["hex"][0]
                                    for half in range(2):
                                        c0 = half * 512
                                        nc.tensor.matmul(
                                            dn[:, c0 : c0 + 512],
                                            lhsT=ones128[:],
                                            rhs=hx0[:, c0 : c0 + 512],
                                            start=True,
                                            stop=False,
                                        )
                                if kt == SHIFT:
                                    po = po_pool.tile(
                                        [128, 512], f32, name="po", tag="po"
                                    )
                                if kt >= SHIFT:
                                    emit_pv(blk, kt - SHIFT, pts, po, tree)
                                # deadline-paced unit pops, kept away from the
                                # block-boundary key-tiles that carry the
                                # previous block's denominator/normalize work
                                npop = 0
                                if blk == 0:
                                    npop = 3 if kt < 14 else 2
                                elif blk < 4:
                                    npop = 0 if kt < 4 else (2 if kt < 14 else 1)
                                elif blk < 7:
                                    npop = 1 if kt in (5, 7, 9, 11) else 0
                                for _ in range(npop):
                                    if blk < 4 and units:
                                        units.pop()()
                                    elif late_units:
                                        late_units.pop()()
                            st["pending"] = (blk, pts, po, tree)
                    # flush the last block (denA already ran at kt13)
                    tail_a1(*st["pending"])
                    tail_a2(*st["pending"])
                    tail_c(*st["pending"])
                    st["pending"] = None
                    while units:
                        units.pop()()
                    while late_units:
                        late_units.pop()()
                    for buf_in, buf_out in ((a2a_inA, a2a_outA), (a2a_inB, a2a_outB)):
                        nc.gpsimd.collective_compute(
                            "AllToAll",
                            mybir.AluOpType.bypass,
                            replica_groups=[list(range(NCORES))],
                            ins=[buf_in[:].opt()],
                            outs=[buf_out[:].opt()],
                        )

            # ---------------- phase 3: output projection ---------------
            with (
                tc.tile_pool(name="ots", bufs=1) as ots_pool,
                tc.tile_pool(name="psout", bufs=1, space="PSUM") as ps_out,
                tc.tile_pool(name="outt", bufs=2) as out_pool,
            ):
                pouts = [
                    ps_out.tile([128, 256], f32, name=f"pout{m}", tag=f"po{m}")
                    for m in range(8)
                ]
                for half, buf_out in ((0, a2a_outA), (1, a2a_outB)):
                    for k in range(KT):
                        o_t = ots_pool.tile(
                            [128, RPC // 2], bf16, name=f"oTs_{k}", tag=f"ot{k}"
                        )
                        nc.sync.dma_start(
                            out=o_t[:], in_=buf_out[k * 128 : (k + 1) * 128, :]
                        )
                        last = k == KT - 1
                        for m in range(8):
                            nc.tensor.matmul(
                                pouts[m][:],
                                lhsT=wo_sb[k][:, m * 128 : (m + 1) * 128],
                                rhs=o_t[:],
                                start=(k == 0),
                                stop=last,
                            )
                            if last:
                                o_sb = out_pool.tile(
                                    [128, 256], bf16, name="o_sb", tag="o_sb"
                                )
                                nc.vector.tensor_scalar_add(
                                    out=o_sb[:],
                                    in0=pouts[m][:],
                                    scalar1=bias_o[:, m : m + 1],
                                )
                                nc.sync.dma_start(
                                    out=out[
                                        m * 128 : (m + 1) * 128,
                                        half * 256 : (half + 1) * 256,
                                    ],
                                    in_=o_sb[:],
                                )

    nc.compile()
    return nc


def _get_graph():
    if "nc" not in _CACHED:
        _CACHED["nc"] = _build_graph()
    return _CACHED["nc"]


def _make_in_maps(x, wqkv, bqkv, wo, bo):
    bf = ml_dtypes.bfloat16
    x2 = np.asarray(x, dtype=np.float32).reshape(R, DIM)
    xt = np.ascontiguousarray(x2.T.astype(bf))  # [dim, b*s] bf16
    wqkv = np.asarray(wqkv, dtype=np.float32)
    bqkv = np.asarray(bqkv, dtype=np.float32)
    wo16 = np.ascontiguousarray(np.asarray(wo, dtype=np.float32).astype(bf))
    bo_f = np.ascontiguousarray(np.asarray(bo, dtype=np.float32).reshape(8, 128))

    in_maps = []
    for c in range(NCORES):
        w_s = np.ascontiguousarray(
            np.concatenate(
                [
                    wqkv[:, c * FPC : (c + 1) * FPC],
                    wqkv[:, DIM + c * FPC : DIM + (c + 1) * FPC],
                    wqkv[:, 2 * DIM + c * FPC : 2 * DIM + (c + 1) * FPC],
                ],
                axis=1,
            ).astype(bf)
        )
        b_s = np.ascontiguousarray(
            np.stack(
                [
                    bqkv[c * FPC : (c + 1) * FPC],
                    bqkv[DIM + c * FPC : DIM + (c + 1) * FPC],
                    bqkv[2 * DIM + c * FPC : 2 * DIM + (c + 1) * FPC],
                ],
                axis=0,
            )
        )
        in_maps.append({"xt": xt, "wqkv": w_s, "bqkv": b_s, "wo": wo16, "bo": bo_f})
    return in_maps


def kernel(x, wqkv, bqkv, wo, bo):
    from concourse.bass_utils import run_bass_kernel_spmd

    nc = _get_graph()
    in_maps = _make_in_maps(x, wqkv, bqkv, wo, bo)
    res = run_bass_kernel_spmd(nc, in_maps, core_ids=list(range(NCORES)))
    outs = [res.results[c]["out"] for c in range(NCORES)]  # each [1024, 512]
    full = np.concatenate([o.T for o in outs], axis=0)  # [4096, 1024]
    return np.ascontiguousarray(full.reshape(B, S, DIM)).astype(np.float32)
